# revision 37
# baseline (speedup 1.0000x reference)
"""Radon transform (bilinear grid-sample + row-sum) on 8 TRN2 NeuronCores.

Tile-gather design: each angle's sample lines are walked in a frame variant
(identity / transpose / +-diagonal shear) chosen so the line advances < 0.59
columns per row.  The frame is stored column-sliced across each 16-partition
GPSIMD group (partition = (col mod 8, batch)); one ap_gather index fetches a
d x 8 pixel tile (d rows x 8 adjacent columns x 2 batches) in one shot, so
the per-index RD_CMD latency (the kernel bottleneck) is amortized over a
whole 2-D tile instead of a single bilinear tap pair.  All 8 groups hold
identical slabs; a bin's tiles are dealt round-robin to groups.  DVE applies
precomputed weights + segment-reduce per sinogram bin; a ones-ish matmul
sums partitions; per-slot DMA returns [2, 368] sinogram columns.

All gather indices / weights are input-independent and precomputed on host.
"""
import math
import os
import sys
from contextlib import ExitStack

import numpy as np

sys.path.insert(0, "/opt/trn_rl_repo")

import ml_dtypes  # noqa: E402

BF16 = ml_dtypes.bfloat16

# ─── geometry constants (hardcoded for 256x256, 180 angles, batch 2) ───
N_ANGLES = 180
IMG_SIZE = 256
BATCH = 2
S = int(math.ceil(math.sqrt(2.0) * IMG_SIZE))  # 363

ROWS = 384            # slab rows (multiple of 32)
XT = 92               # col-tiles per partition slice
NELS = XT * ROWS      # 35328 elements per partition
NCOL = XT * 8         # 736 layout columns
SXPAD = 368           # padded bin count
SLOTS = 24            # slot 0 is a dummy pipeline-priming slot
DUMMY_LSEG = 4
NGROUP = 8
D_CHOICES = (8, 16, 24, 32, 48)
# per-d bin chunking: nxg multiples of 16 (index wrap), sized so
# nxg*lseg*d stays under the SBUF chunk-volume cap even at large lseg
D_CHUNKS = {
    8: [64, 64, 64, 64, 64, 48],
    16: [64, 64, 64, 64, 64, 48],
    24: [48, 48, 48, 48, 48, 48, 48, 32],
    32: [48, 48, 48, 48, 48, 48, 48, 32],
    48: [32] * 11 + [16],
}
CHUNK_NXG = D_CHUNKS[16]  # dummy-slot scheme
MAXVOL = 8192
# hardware-measured per-index gather cost (cycles) by block depth d
GATHER_CYC = {2: 33.0, 8: 40.0, 16: 39.5, 24: 56.8, 32: 62.3, 48: 76.0}
DVE_NS_PER_EL = 1.70     # measured incl. per-instruction overhead
CHUNK_OVERHEAD_NS = 2100  # per-chunk gather dispatch/idx-load/sync cost

# angle classes: (variant, angle list). variant row/col maps:
#   v0: row=Y+1, col=X+1            (|tan| <= tan22.5, theta near 0/180)
#   v1: row=X+1, col=Y+1            (theta near 90)
#   v2a: row=Y+1, col=X-Y+368       (22.5 < th <= 45)
#   v2b: row=X+1, col=X-Y+368       (45 < th < 67.5)
#   v3a: row=X+1, col=X+Y+2         (112.5 < th <= 135)
#   v3b: row=Y+1, col=X+Y+2         (135 < th < 157.5)
CORE_SPECS = [
    ("v0", list(range(0, 23))),
    ("v0", list(range(158, 180))),
    ("v2a", list(range(23, 46))),
    ("v2b", list(range(46, 68))),
    ("v1", list(range(68, 91))),
    ("v1", list(range(91, 113))),
    ("v3a", list(range(113, 136))),
    ("v3b", list(range(136, 158))),
]


def _angle_tables(k):
    theta = np.float32(k) * np.float32(np.pi / N_ANGLES)
    c = np.cos(theta, dtype=np.float32)
    s = np.sin(theta, dtype=np.float32)
    lin = np.linspace(-1.0, 1.0, S, dtype=np.float32)
    x = lin[None, :]
    y = lin[:, None]
    gx = c * x + s * y
    gy = -s * x + c * y
    ix = ((gx + np.float32(1.0)) * np.float32(0.5) * np.float32(S - 1)).astype(np.float32)
    iy = ((gy + np.float32(1.0)) * np.float32(0.5) * np.float32(S - 1)).astype(np.float32)
    x0 = np.floor(ix)
    y0 = np.floor(iy)
    wx = ix - x0
    wy = iy - y0
    return y0.astype(np.int64), x0.astype(np.int64), wx, wy


def _variant_rc(var, Y, X):
    if var == "v0":
        return Y + 1, X + 1
    if var == "v1":
        return X + 1, Y + 1
    if var == "v2a":
        return Y + 1, X - Y + 368
    if var == "v2b":
        return X + 1, X - Y + 368
    if var == "v3a":
        return X + 1, X + Y + 2
    return Y + 1, X + Y + 2  # v3b


def _angle_taps(k, var):
    """Flat arrays over valid taps: bin, samp, row, col, w (f32)."""
    y0, x0, wx, wy = _angle_tables(k)
    kk = np.broadcast_to(np.arange(S, dtype=np.int64)[:, None], (S, S))
    bb = np.broadcast_to(np.arange(S, dtype=np.int64)[None, :], (S, S))
    outs = []
    for dy in (0, 1):
        for dx in (0, 1):
            Y = y0 + dy
            X = x0 + dx
            w = (wy if dy else (1.0 - wy)) * (wx if dx else (1.0 - wx))
            v = (Y >= 0) & (Y < S) & (X >= 0) & (X < S)
            r, c = _variant_rc(var, Y, X)
            outs.append((bb[v], kk[v], r[v], c[v], w[v].astype(np.float32)))
    bins = np.concatenate([o[0] for o in outs])
    samp = np.concatenate([o[1] for o in outs])
    row = np.concatenate([o[2] for o in outs])
    col = np.concatenate([o[3] for o in outs])
    w = np.concatenate([o[4] for o in outs])
    return bins, samp, row, col, w


def _angle_tiles(k, var, d):
    """Per-bin ordered unique tiles + per-tap (group, slot, channel, elem).

    Returns dict with:
      lsegs[6]      per-chunk max segment length (ceil(ntiles/8))
      (after _plan pass B) idx/w scatter info
    """
    bins, samp, row, col, w = _angle_taps(k, var)
    rb = row // d
    xt = col // 8
    jc = col % 8
    e = row % d
    nrb = ROWS // d
    tile = xt * nrb + rb                       # block index in slab (< 4416)
    key = bins * (XT * nrb) + tile
    order = np.lexsort((samp, key))
    key_s = key[order]
    samp_s = samp[order]
    uk, first = np.unique(key_s, return_index=True)
    firstk = samp_s[first]                     # first sample touching tile
    ubin = uk // (XT * nrb)
    utile = uk % (XT * nrb)
    # order tiles within bin by first-sample
    o2 = np.lexsort((firstk, ubin))
    ubin2 = ubin[o2]
    utile2 = utile[o2]
    starts = np.searchsorted(ubin2, np.arange(S + 1))
    pos = np.arange(len(ubin2)) - starts[ubin2]
    ntiles = np.diff(starts)                   # tiles per bin
    # invert o2: for each unique-key row, its (group, t)
    grp_u = np.empty(len(uk), np.int64)
    t_u = np.empty(len(uk), np.int64)
    grp_u[o2] = pos % NGROUP
    t_u[o2] = pos // NGROUP
    # per-chunk lsegs
    edges = np.cumsum([0] + CHUNK_NXG)
    lsegs = []
    for ci in range(len(CHUNK_NXG)):
        lo, hi = edges[ci], min(edges[ci + 1], S)
        nt = ntiles[lo:hi] if hi > lo else np.zeros(1, np.int64)
        lsegs.append(int(np.ceil(nt.max() / NGROUP)) if len(nt) else 1)
    return dict(lsegs=[max(l, 1) for l in lsegs], uk=uk, ubin2=ubin2,
                utile2=utile2, grp_u=grp_u, t_u=t_u, key=key, jc=jc, e=e,
                w=w, bins=bins)


def _gather_cyc(nidx, d):
    return nidx * GATHER_CYC[d]


_PLAN_CACHE = {}


def _get_plan():
    if "plan" in _PLAN_CACHE:
        return _PLAN_CACHE["plan"]
    # pass A: per (core, angle, d): lsegs + cost -> per-core angle order,
    # global d_table / lseg_table.  All d are multiples of 8, so tile sets
    # for every d derive from one unique pass at d=8.
    info = {}
    for ci, (var, angles) in enumerate(CORE_SPECS):
        for k in angles:
            bins, samp, row, col, w = _angle_taps(k, var)
            key8 = (bins * XT + col // 8) * 48 + row // 8
            u8 = np.unique(key8)
            ubin8 = u8 // (48 * XT)
            uxt8 = (u8 // 48) % XT
            urb8 = u8 % 48
            per_d = {}
            for d in D_CHOICES:
                m = d // 8
                kd = (ubin8 * XT + uxt8) * 48 + urb8 // m
                ud = np.unique(kd)
                nt = np.bincount(ud // (48 * XT), minlength=S)
                nxgs = D_CHUNKS[d]
                edges = np.cumsum([0] + nxgs)
                lsegs = []
                for cix in range(len(nxgs)):
                    lo, hi = edges[cix], min(edges[cix + 1], S)
                    mx = nt[lo:hi].max() if hi > lo else 0
                    lsegs.append(max(int(np.ceil(mx / NGROUP)), 1))
                nidx = sum(nxg * l for nxg, l in zip(nxgs, lsegs))
                gat_ns = _gather_cyc(nidx, d) / 1.2
                dve_ns = nidx * d * DVE_NS_PER_EL
                per_d[d] = (lsegs, max(gat_ns, dve_ns), gat_ns, dve_ns)
            info[k] = per_d
    # per-core angle order: hardest first (by best-d cost)
    core_order = []
    for ci, (var, angles) in enumerate(CORE_SPECS):
        best = {k: min(v[d][1] for d in D_CHOICES) for k, v in
                ((k, info[k]) for k in angles)}
        core_order.append(sorted(angles, key=lambda k: -best[k]))

    # per-slot, per-d: cross-core max lsegs -> (gather_ns, dve_ns)
    def slot_cost(si, d):
        ls = [1] * len(D_CHUNKS[d])
        for ci in range(8):
            if si < len(core_order[ci]):
                al = info[core_order[ci][si]][d][0]
                ls = [max(a, b) for a, b in zip(ls, al)]
        nidx = sum(nxg * l for nxg, l in zip(D_CHUNKS[d], ls))
        gat = _gather_cyc(nidx, d) / 1.2 + CHUNK_OVERHEAD_NS * len(D_CHUNKS[d])
        return ls, gat, nidx * d * DVE_NS_PER_EL

    NSLOT_A = SLOTS - 1   # angle slots (slot 0 is the dummy priming slot)
    cost = {}
    for si in range(NSLOT_A):
        for d in D_CHOICES:
            ls, gat, dve = slot_cost(si, d)
            # cap per-chunk gathered volume so 4-deep DMA buffers fit SBUF
            if max(nxg * l * d for nxg, l in zip(D_CHUNKS[d], ls)) > MAXVOL:
                gat = dve = 1e18
            cost[(si, d)] = (ls, gat, dve)
    # choose d per slot minimizing max(total gather, total dve)
    d_table = [min(D_CHOICES, key=lambda d: cost[(si, d)][1])
               for si in range(NSLOT_A)]
    while True:
        G = sum(cost[(si, d_table[si])][1] for si in range(NSLOT_A))
        V = sum(cost[(si, d_table[si])][2] for si in range(NSLOT_A))
        best_move = None
        for si in range(NSLOT_A):
            for d in D_CHOICES:
                if d == d_table[si]:
                    continue
                g2 = G - cost[(si, d_table[si])][1] + cost[(si, d)][1]
                v2 = V - cost[(si, d_table[si])][2] + cost[(si, d)][2]
                m = max(g2, v2)
                if best_move is None or m < best_move[0]:
                    best_move = (m, si, d)
        if best_move is None or best_move[0] >= max(G, V) - 1.0:
            break
        d_table[best_move[1]] = best_move[2]
    lseg_table = [cost[(si, d_table[si])][0] for si in range(NSLOT_A)]
    d_table = [16] + d_table
    lseg_table = [[DUMMY_LSEG] * len(CHUNK_NXG)] + lseg_table
    # stream layout: per slot, per chunk: cn = nxg * lseg
    chunks = []
    o16 = ow = 0
    for si in range(SLOTS):
        d = d_table[si]
        nxgs = D_CHUNKS[d]
        for cidx, nxg in enumerate(nxgs):
            L = lseg_table[si][cidx]
            cn = nxg * L
            chunks.append(dict(si=si, cidx=cidx, d=d, L=L, cn=cn,
                               xoff=sum(nxgs[:cidx]), nxg=nxg,
                               o16=o16, ow=ow))
            o16 += cn // 16
            ow += cn * d
    tot16, totw = o16, ow
    maxcn = max(ch["cn"] for ch in chunks)
    maxels = max(ch["cn"] * ch["d"] for ch in chunks)

    # pass B: build per-core idx blobs [128, tot16] + w blobs [64, totw]
    core_idx = []
    core_w = []
    for ci, (var, angles) in enumerate(CORE_SPECS):
        idx_blob = np.zeros((128, tot16), np.int16)
        w_blob = np.zeros((64, totw), np.float32)
        for si in range(SLOTS):
            d = d_table[si]
            if si == 0 or si - 1 >= len(core_order[ci]):
                continue
            k = core_order[ci][si - 1]
            a = _angle_tiles(k, var, d)
            lsegs = lseg_table[si]
            sch = [c for c in chunks if c["si"] == si]
            nxgs = D_CHUNKS[d]
            edges = np.cumsum([0] + nxgs)
            # per-(bin,group) stream position of tile t:
            #   spos = chunk.o16*16 + (bin-lo)*L + t   (per-group stream)
            ub, ut = a["ubin2"], a["utile2"]
            # grp/t in o2 order: pos within bin
            starts = np.searchsorted(ub, np.arange(S + 1))
            pos = np.arange(len(ub)) - starts[ub]
            grp = pos % NGROUP
            tt = pos // NGROUP
            cid = np.searchsorted(edges, ub, side="right") - 1
            L_arr = np.array([lsegs[c] for c in range(len(nxgs))])
            off_arr = np.array([sch[c]["o16"] * 16 for c in range(len(nxgs))])
            assert np.all(tt < L_arr[cid]), (ci, si, k)
            spos = off_arr[cid] + (ub - edges[cid]) * L_arr[cid] + tt
            # scatter idx values: stream for group g wrapped into
            # partitions 16g..16g+15: idx[16g + (p%16), p//16] = val
            # default padding: repeat previous valid idx (avoid addr jumps)
            sv = np.zeros((NGROUP, tot16 * 16), np.int16)
            filled = np.zeros((NGROUP, tot16 * 16), bool)
            sv[grp, spos] = ut
            filled[grp, spos] = True
            # forward-fill padding within this slot's range
            lo16, hi16 = sch[0]["o16"] * 16, (sch[-1]["o16"] + sch[-1]["cn"] // 16) * 16
            for g in range(NGROUP):
                seg = sv[g, lo16:hi16]
                fil = filled[g, lo16:hi16]
                idxs = np.where(fil, np.arange(len(seg)), 0)
                np.maximum.accumulate(idxs, out=idxs)
                sv[g, lo16:hi16] = seg[idxs]
            # wrap into idx_blob
            for g in range(NGROUP):
                st = sv[g, lo16:hi16]
                wrap = st.reshape(-1, 16).T
                idx_blob[16 * g:16 * g + 16, lo16 // 16:hi16 // 16] = wrap
            # weights: per tap: blob row = 8*grp_tap + jc, col = spos*d + e
            tap_key = a["key"]
            urow = np.searchsorted(a["uk"], tap_key)
            tap_grp = a["grp_u"][urow]
            tap_t = a["t_u"][urow]
            tap_bin = a["bins"]
            tap_cid = np.searchsorted(edges, tap_bin, side="right") - 1
            tap_spos = (off_arr[tap_cid] + (tap_bin - edges[tap_cid])
                        * L_arr[tap_cid] + tap_t)
            wrow = 8 * tap_grp + a["jc"]
            ow_arr = np.array([sch[c]["ow"] for c in range(len(nxgs))])
            o16_arr = np.array([sch[c]["o16"] * 16 for c in range(len(nxgs))])
            wcol = ow_arr[tap_cid] + (tap_spos - o16_arr[tap_cid]) * d + a["e"]
            np.add.at(w_blob, (wrow, wcol), a["w"])
        core_idx.append(idx_blob)
        core_w.append(w_blob.astype(BF16))

    sel = np.zeros((128, 2), np.float32)
    for p in range(128):
        sel[p, p % 2] = 1.0
    plan = dict(d_table=d_table, lseg_table=lseg_table, chunks=chunks,
                tot16=tot16, totw=totw, maxcn=maxcn, maxels=maxels,
                core_idx=core_idx, core_w=core_w, sel=sel,
                core_order=core_order)
    _PLAN_CACHE["plan"] = plan
    return plan


def _build_slab(image, var):
    """[128, NELS] bf16: partition p=(g,jc,b): cols ≡ jc mod 8 of variant
    frame, batch b; element idx = xt*ROWS + row."""
    img = np.asarray(image, np.float32)[:, 0]
    # padded image: 256 -> 363
    pad_total = S - IMG_SIZE
    pb = pad_total // 2
    pimg = np.zeros((BATCH, S, S), np.float32)
    pimg[:, pb:pb + IMG_SIZE, pb:pb + IMG_SIZE] = img
    Yg, Xg = np.meshgrid(np.arange(S), np.arange(S), indexing="ij")
    r, c = _variant_rc(var, Yg, Xg)
    frame = np.zeros((BATCH, ROWS, NCOL), np.float32)
    frame[:, r, c] = pimg
    # slice: [16, XT, ROWS]
    slab16 = np.zeros((16, NELS), np.float32)
    for jc in range(8):
        cols = frame[:, :, jc::8]              # [B, ROWS, XT]
        sl = np.transpose(cols, (0, 2, 1)).reshape(BATCH, -1)  # xt-major rows
        for b in range(BATCH):
            slab16[2 * jc + b] = sl[b]
    return np.tile(slab16, (8, 1)).astype(BF16)


_PROG_CACHE = {}


def _build_program(plan):
    if "prog" in _PROG_CACHE:
        return _PROG_CACHE["prog"]
    import concourse.bass as bass
    import concourse.mybir as mybir
    from concourse import library_config

    chunks = plan["chunks"]
    maxcn = plan["maxcn"]
    maxels = plan["maxels"]

    nc = bass.Bass()
    slab_d = nc.declare_dram_parameter("slab", [128, NELS],
                                       mybir.dt.bfloat16, isOutput=False)
    idx_d = nc.declare_dram_parameter("idx", [128, plan["tot16"]],
                                      mybir.dt.int16, isOutput=False)
    w_d = nc.declare_dram_parameter("w", [64, plan["totw"]],
                                    mybir.dt.bfloat16, isOutput=False)
    sel_d = nc.declare_dram_parameter("sel", [128, 2], mybir.dt.float32,
                                      isOutput=False)
    out_d = nc.declare_dram_parameter("out", [SLOTS, 2, SXPAD],
                                      mybir.dt.float32, isOutput=True)

    ctx = ExitStack()
    with ctx:
        slab_t = ctx.enter_context(nc.sbuf_tensor([128, NELS], mybir.dt.bfloat16))
        idx_ts = [ctx.enter_context(nc.sbuf_tensor(f"idx{i}", [128, maxcn // 16], mybir.dt.int16)) for i in range(4)]
        w_ts = [ctx.enter_context(nc.sbuf_tensor(f"w{i}", [128, maxels], mybir.dt.bfloat16)) for i in range(4)]
        g_ts = [ctx.enter_context(nc.sbuf_tensor(f"g{i}", [128, maxels], mybir.dt.bfloat16)) for i in range(2)]
        p_t = ctx.enter_context(nc.sbuf_tensor([128, maxels], mybir.dt.bfloat16))
        r_ts = [ctx.enter_context(nc.sbuf_tensor(f"r{i}", [128, SXPAD], mybir.dt.float32)) for i in range(2)]
        sel_t = ctx.enter_context(nc.sbuf_tensor([128, 2], mybir.dt.float32))
        vscr_t = ctx.enter_context(nc.sbuf_tensor([128, 2], mybir.dt.float32))
        sino_ts = [ctx.enter_context(nc.sbuf_tensor(f"sino{i}", [2, SXPAD], mybir.dt.float32)) for i in range(2)]
        psum_ts = [ctx.enter_context(nc.psum_tensor(f"ps{i}", [2, SXPAD], mybir.dt.float32)) for i in range(2)]
        s_in = ctx.enter_context(nc.semaphore("s_in"))
        s_dma = ctx.enter_context(nc.semaphore("s_dma"))
        s_g = ctx.enter_context(nc.semaphore("s_g"))
        s_v = ctx.enter_context(nc.semaphore("s_v"))
        s_mm = ctx.enter_context(nc.semaphore("s_mm"))
        s_cp = ctx.enter_context(nc.semaphore("s_cp"))
        s_od = ctx.enter_context(nc.semaphore("s_od"))
        block = ctx.enter_context(nc.Block())

        slot_end = [0] * SLOTS
        for n, ch in enumerate(chunks):
            slot_end[ch["si"]] = n + 1

        @block.sync
        def _(sync):
            sync.dma_start(out=sel_t[:], in_=sel_d[:]).then_inc(s_in, 16)
            sync.dma_start(out=slab_t[:], in_=slab_d[:]).then_inc(s_in, 16)
            for n, ch in enumerate(chunks):
                # 4-deep prefetch: buffer n%4 was last used by chunk n-4
                # (idx read by gather n-4, w read by vector n-4); the deep
                # pipeline also gives cold-start DMAs time to actually land
                # (completion semaphores fire early).
                if n > 3:
                    sync.wait_ge(s_g, n - 3)
                    sync.wait_ge(s_v, n - 3)
                sync.dma_start(
                    out=idx_ts[n % 4][:, :ch["cn"] // 16],
                    in_=idx_d[:, ch["o16"]:ch["o16"] + ch["cn"] // 16],
                ).then_inc(s_dma, 16)
                cnd = ch["cn"] * ch["d"]
                wsrc = (w_d[:, ch["ow"]:ch["ow"] + cnd]
                        .unsqueeze(1).broadcast_to([64, 2, cnd]))
                sync.dma_start(out=w_ts[n % 4][:, :cnd], in_=wsrc).then_inc(s_dma, 16)

        @block.gpsimd
        def _(g):
            g.load_library(library_config.ap_gather)
            g.wait_ge(s_in, 32)
            g.wait_ge(s_dma, 32)
            # warmup: amortize ext-isa first-call cost + preamble margin
            ch0 = chunks[0]
            d0 = ch0["d"]
            for _ in range(2):
                g.ap_gather(
                    g_ts[1][:, :ch0["cn"] * d0].rearrange("p (n d) -> p n d", d=d0),
                    slab_t[:].rearrange("p (n d) -> p n d", d=d0),
                    idx_ts[0][:, :ch0["cn"] // 16],
                    channels=128, num_elems=NELS // d0, d=d0, num_idxs=ch0["cn"],
                )
            for n, ch in enumerate(chunks):
                d = ch["d"]
                g.wait_ge(s_dma, 32 * (n + 1))
                if n > 1:
                    g.wait_ge(s_v, n - 1)
                g.ap_gather(
                    g_ts[n % 2][:, :ch["cn"] * d].rearrange("p (n d) -> p n d", d=d),
                    slab_t[:].rearrange("p (n d) -> p n d", d=d),
                    idx_ts[n % 4][:, :ch["cn"] // 16],
                    channels=128, num_elems=NELS // d, d=d, num_idxs=ch["cn"],
                ).then_inc(s_g, 1)

        @block.vector
        def _(v):
            for n, ch in enumerate(chunks):
                v.wait_ge(s_g, n + 1)
                if ch["cidx"] == 0 and ch["si"] > 1:
                    v.wait_ge(s_mm, ch["si"] - 1)
                cnd = ch["cn"] * ch["d"]
                v.tensor_mul(p_t[:, :cnd], g_ts[n % 2][:, :cnd],
                             w_ts[n % 4][:, :cnd])
                rdst = r_ts[ch["si"] % 2]
                v.tensor_reduce(
                    out=rdst[:, ch["xoff"]:ch["xoff"] + ch["nxg"]],
                    in_=p_t[:, :cnd].rearrange(
                        "p (x l) -> p x l", l=ch["L"] * ch["d"]),
                    axis=mybir.AxisListType.X,
                    op=mybir.AluOpType.add,
                )
                v.tensor_copy(vscr_t[:, :1],
                              rdst[:, ch["xoff"]:ch["xoff"] + 1]).then_inc(s_v, 1)

        @block.tensor
        def _(t):
            t.wait_ge(s_in, 32)
            for si in range(SLOTS):
                t.wait_ge(s_v, slot_end[si])
                if si > 1:
                    t.wait_ge(s_cp, si - 1)
                t.matmul(psum_ts[si % 2][:], sel_t[:], r_ts[si % 2][:],
                         start=True, stop=True).then_inc(s_mm, 1)

        @block.scalar
        def _(sc):
            for si in range(SLOTS):
                sc.wait_ge(s_mm, si + 1)
                if si > 1:
                    sc.wait_ge(s_od, 16 * (si - 1))  # sino buf freed by DMA
                sc.copy(sino_ts[si % 2][:], psum_ts[si % 2][:]).then_inc(s_cp, 1)
                sc.dma_start(out=out_d[si], in_=sino_ts[si % 2][:]
                             ).then_inc(s_od, 16)
            sc.wait_ge(s_od, 16 * SLOTS)

    import concourse.mybir as mybir2
    mybir2.codegen_inst_isa_subclasses(nc)
    _PROG_CACHE["prog"] = nc
    return nc


def kernel(image):
    image = np.asarray(image, np.float32)
    assert image.shape == (BATCH, 1, IMG_SIZE, IMG_SIZE)
    plan = _get_plan()
    nc = _build_program(plan)

    from concourse.bass_utils import run_bass_kernel_spmd

    in_maps = []
    for ci, (var, angles) in enumerate(CORE_SPECS):
        in_maps.append({
            "slab": _build_slab(image, var),
            "idx": plan["core_idx"][ci],
            "w": plan["core_w"][ci],
            "sel": plan["sel"],
        })

    trace = bool(os.environ.get("RADON_TRACE"))
    if trace:
        _install_profhook()
    res = run_bass_kernel_spmd(nc, in_maps, list(range(8)), trace=trace)
    if trace:
        kernel.last_exec_time_ns = res.exec_time_ns

    sino = np.zeros((BATCH, 1, S, N_ANGLES), np.float32)
    for ci in range(8):
        o = res.results[ci]["out"]  # [SLOTS, 2, SXPAD]; slot 0 is dummy
        for si, k in enumerate(plan["core_order"][ci]):
            sino[:, 0, :, k] = o[si + 1, :, :S]
    return sino


def _install_profhook():
    import types
    if "antenv.axon_hooks" in sys.modules:
        return
    try:
        from trn_agent_boot.trn_boot import _ntff_profile_via_ctypes
        hook = _ntff_profile_via_ctypes("/opt/axon/libaxon_pjrt.so")
    except Exception:
        hook = None
    mod = types.ModuleType("antenv.axon_hooks")
    mod._hook = hook
    mod.set_axon_ntff_profile_hook = lambda h: setattr(mod, "_hook", h)
    mod.get_axon_ntff_profile_hook = lambda: mod._hook
    sys.modules["antenv.axon_hooks"] = mod
    import antenv
    antenv.axon_hooks = mod


if __name__ == "__main__":
    img = np.load("/tmp/ref_image.npy")
    out = kernel(image=img)
    exp = np.load("/tmp/ref_expected.npy")
    err = np.linalg.norm(out - exp) / np.linalg.norm(exp)
    print("kernel rel err:", err)


# revision 39
# speedup vs baseline: 1.0175x; 1.0175x over previous
"""Radon transform (bilinear grid-sample + row-sum) on 8 TRN2 NeuronCores.

Tile-gather design: each angle's sample lines are walked in a frame variant
(identity / transpose / +-diagonal shear) chosen so the line advances < 0.59
columns per row.  The frame is stored column-sliced across each 16-partition
GPSIMD group (partition = (col mod 8, batch)); one ap_gather index fetches a
d x 8 pixel tile (d rows x 8 adjacent columns x 2 batches) in one shot, so
the per-index RD_CMD latency (the kernel bottleneck) is amortized over a
whole 2-D tile instead of a single bilinear tap pair.  All 8 groups hold
identical slabs; a bin's tiles are dealt round-robin to groups.  DVE applies
precomputed weights + segment-reduce per sinogram bin; a ones-ish matmul
sums partitions; per-slot DMA returns [2, 368] sinogram columns.

All gather indices / weights are input-independent and precomputed on host.
"""
import math
import os
import sys
from contextlib import ExitStack

import numpy as np

sys.path.insert(0, "/opt/trn_rl_repo")

import ml_dtypes  # noqa: E402

BF16 = ml_dtypes.bfloat16

# ─── geometry constants (hardcoded for 256x256, 180 angles, batch 2) ───
N_ANGLES = 180
IMG_SIZE = 256
BATCH = 2
S = int(math.ceil(math.sqrt(2.0) * IMG_SIZE))  # 363

ROWS = 384            # slab rows (multiple of 32)
XT = 92               # col-tiles per partition slice
NELS = XT * ROWS      # 35328 elements per partition
NCOL = XT * 8         # 736 layout columns
SXPAD = 368           # padded bin count
SLOTS = 24            # slot 0 is a dummy pipeline-priming slot
DUMMY_LSEG = 4
NGROUP = 8
D_CHOICES = (8, 16, 24, 32, 48)
# per-d bin chunking: nxg multiples of 16 (index wrap), sized so
# nxg*lseg*d stays under the SBUF chunk-volume cap even at large lseg
D_CHUNKS = {
    8: [64, 64, 64, 64, 64, 48],
    16: [64, 64, 64, 64, 64, 48],
    24: [64, 64, 64, 64, 64, 48],
    32: [64, 64, 64, 64, 64, 48],
    48: [64, 64, 64, 64, 64, 48],
}
CHUNK_NXG = D_CHUNKS[16]  # dummy-slot scheme
MAXVOL = 8192
# hardware-measured per-index gather cost (cycles) by block depth d
GATHER_CYC = {2: 33.0, 8: 40.0, 16: 39.5, 24: 56.8, 32: 62.3, 48: 76.0}
DVE_NS_PER_EL = 1.33     # mult (2.2 el/ns) + segment reduce (1.13 el/ns)
CHUNK_OVERHEAD_NS = 0     # per-chunk gather dispatch/idx-load/sync cost

# angle classes: (variant, angle list). variant row/col maps:
#   v0: row=Y+1, col=X+1            (|tan| <= tan22.5, theta near 0/180)
#   v1: row=X+1, col=Y+1            (theta near 90)
#   v2a: row=Y+1, col=X-Y+368       (22.5 < th <= 45)
#   v2b: row=X+1, col=X-Y+368       (45 < th < 67.5)
#   v3a: row=X+1, col=X+Y+2         (112.5 < th <= 135)
#   v3b: row=Y+1, col=X+Y+2         (135 < th < 157.5)
CORE_SPECS = [
    ("v0", list(range(0, 23))),
    ("v0", list(range(158, 180))),
    ("v2a", list(range(23, 46))),
    ("v2b", list(range(46, 68))),
    ("v1", list(range(68, 91))),
    ("v1", list(range(91, 113))),
    ("v3a", list(range(113, 136))),
    ("v3b", list(range(136, 158))),
]


def _angle_tables(k):
    theta = np.float32(k) * np.float32(np.pi / N_ANGLES)
    c = np.cos(theta, dtype=np.float32)
    s = np.sin(theta, dtype=np.float32)
    lin = np.linspace(-1.0, 1.0, S, dtype=np.float32)
    x = lin[None, :]
    y = lin[:, None]
    gx = c * x + s * y
    gy = -s * x + c * y
    ix = ((gx + np.float32(1.0)) * np.float32(0.5) * np.float32(S - 1)).astype(np.float32)
    iy = ((gy + np.float32(1.0)) * np.float32(0.5) * np.float32(S - 1)).astype(np.float32)
    x0 = np.floor(ix)
    y0 = np.floor(iy)
    wx = ix - x0
    wy = iy - y0
    return y0.astype(np.int64), x0.astype(np.int64), wx, wy


def _variant_rc(var, Y, X):
    if var == "v0":
        return Y + 1, X + 1
    if var == "v1":
        return X + 1, Y + 1
    if var == "v2a":
        return Y + 1, X - Y + 368
    if var == "v2b":
        return X + 1, X - Y + 368
    if var == "v3a":
        return X + 1, X + Y + 2
    return Y + 1, X + Y + 2  # v3b


def _angle_taps(k, var):
    """Flat arrays over valid taps: bin, samp, row, col, w (f32)."""
    y0, x0, wx, wy = _angle_tables(k)
    kk = np.broadcast_to(np.arange(S, dtype=np.int64)[:, None], (S, S))
    bb = np.broadcast_to(np.arange(S, dtype=np.int64)[None, :], (S, S))
    outs = []
    for dy in (0, 1):
        for dx in (0, 1):
            Y = y0 + dy
            X = x0 + dx
            w = (wy if dy else (1.0 - wy)) * (wx if dx else (1.0 - wx))
            v = (Y >= 0) & (Y < S) & (X >= 0) & (X < S)
            r, c = _variant_rc(var, Y, X)
            outs.append((bb[v], kk[v], r[v], c[v], w[v].astype(np.float32)))
    bins = np.concatenate([o[0] for o in outs])
    samp = np.concatenate([o[1] for o in outs])
    row = np.concatenate([o[2] for o in outs])
    col = np.concatenate([o[3] for o in outs])
    w = np.concatenate([o[4] for o in outs])
    return bins, samp, row, col, w


def _angle_tiles(k, var, d):
    """Per-bin ordered unique tiles + per-tap (group, slot, channel, elem).

    Returns dict with:
      lsegs[6]      per-chunk max segment length (ceil(ntiles/8))
      (after _plan pass B) idx/w scatter info
    """
    bins, samp, row, col, w = _angle_taps(k, var)
    rb = row // d
    xt = col // 8
    jc = col % 8
    e = row % d
    nrb = ROWS // d
    tile = xt * nrb + rb                       # block index in slab (< 4416)
    key = bins * (XT * nrb) + tile
    order = np.lexsort((samp, key))
    key_s = key[order]
    samp_s = samp[order]
    uk, first = np.unique(key_s, return_index=True)
    firstk = samp_s[first]                     # first sample touching tile
    ubin = uk // (XT * nrb)
    utile = uk % (XT * nrb)
    # order tiles within bin by first-sample
    o2 = np.lexsort((firstk, ubin))
    ubin2 = ubin[o2]
    utile2 = utile[o2]
    starts = np.searchsorted(ubin2, np.arange(S + 1))
    pos = np.arange(len(ubin2)) - starts[ubin2]
    ntiles = np.diff(starts)                   # tiles per bin
    # invert o2: for each unique-key row, its (group, t)
    grp_u = np.empty(len(uk), np.int64)
    t_u = np.empty(len(uk), np.int64)
    grp_u[o2] = pos % NGROUP
    t_u[o2] = pos // NGROUP
    # per-chunk lsegs
    edges = np.cumsum([0] + CHUNK_NXG)
    lsegs = []
    for ci in range(len(CHUNK_NXG)):
        lo, hi = edges[ci], min(edges[ci + 1], S)
        nt = ntiles[lo:hi] if hi > lo else np.zeros(1, np.int64)
        lsegs.append(int(np.ceil(nt.max() / NGROUP)) if len(nt) else 1)
    return dict(lsegs=[max(l, 1) for l in lsegs], uk=uk, ubin2=ubin2,
                utile2=utile2, grp_u=grp_u, t_u=t_u, key=key, jc=jc, e=e,
                w=w, bins=bins)


def _gather_cyc(nidx, d):
    return nidx * GATHER_CYC[d]


_PLAN_CACHE = {}


def _get_plan():
    if "plan" in _PLAN_CACHE:
        return _PLAN_CACHE["plan"]
    # pass A: per (core, angle, d): lsegs + cost -> per-core angle order,
    # global d_table / lseg_table.  All d are multiples of 8, so tile sets
    # for every d derive from one unique pass at d=8.
    info = {}
    for ci, (var, angles) in enumerate(CORE_SPECS):
        for k in angles:
            bins, samp, row, col, w = _angle_taps(k, var)
            key8 = (bins * XT + col // 8) * 48 + row // 8
            u8 = np.unique(key8)
            ubin8 = u8 // (48 * XT)
            uxt8 = (u8 // 48) % XT
            urb8 = u8 % 48
            per_d = {}
            for d in D_CHOICES:
                m = d // 8
                kd = (ubin8 * XT + uxt8) * 48 + urb8 // m
                ud = np.unique(kd)
                nt = np.bincount(ud // (48 * XT), minlength=S)
                nxgs = D_CHUNKS[d]
                edges = np.cumsum([0] + nxgs)
                lsegs = []
                for cix in range(len(nxgs)):
                    lo, hi = edges[cix], min(edges[cix + 1], S)
                    mx = nt[lo:hi].max() if hi > lo else 0
                    lsegs.append(max(int(np.ceil(mx / NGROUP)), 1))
                nidx = sum(nxg * l for nxg, l in zip(nxgs, lsegs))
                gat_ns = _gather_cyc(nidx, d) / 1.2
                dve_ns = nidx * d * DVE_NS_PER_EL
                per_d[d] = (lsegs, max(gat_ns, dve_ns), gat_ns, dve_ns)
            info[k] = per_d
    # per-core angle order: hardest first (by best-d cost)
    core_order = []
    for ci, (var, angles) in enumerate(CORE_SPECS):
        best = {k: min(v[d][1] for d in D_CHOICES) for k, v in
                ((k, info[k]) for k in angles)}
        core_order.append(sorted(angles, key=lambda k: -best[k]))

    # per-slot, per-d: cross-core max lsegs -> (gather_ns, dve_ns)
    def slot_cost(si, d):
        ls = [1] * len(D_CHUNKS[d])
        for ci in range(8):
            if si < len(core_order[ci]):
                al = info[core_order[ci][si]][d][0]
                ls = [max(a, b) for a, b in zip(ls, al)]
        nidx = sum(nxg * l for nxg, l in zip(D_CHUNKS[d], ls))
        gat = _gather_cyc(nidx, d) / 1.2 + CHUNK_OVERHEAD_NS * len(D_CHUNKS[d])
        return ls, gat, nidx * d * DVE_NS_PER_EL

    NSLOT_A = SLOTS - 1   # angle slots (slot 0 is the dummy priming slot)
    cost = {}
    for si in range(NSLOT_A):
        for d in D_CHOICES:
            ls, gat, dve = slot_cost(si, d)
            # cap per-chunk gathered volume so 4-deep DMA buffers fit SBUF
            if max(nxg * l * d for nxg, l in zip(D_CHUNKS[d], ls)) > MAXVOL:
                gat = dve = 1e18
            cost[(si, d)] = (ls, gat, dve)
    # choose d per slot minimizing max(total gather, total dve)
    d_table = [min(D_CHOICES, key=lambda d: cost[(si, d)][1])
               for si in range(NSLOT_A)]
    while True:
        G = sum(cost[(si, d_table[si])][1] for si in range(NSLOT_A))
        V = sum(cost[(si, d_table[si])][2] for si in range(NSLOT_A))
        best_move = None
        for si in range(NSLOT_A):
            for d in D_CHOICES:
                if d == d_table[si]:
                    continue
                g2 = G - cost[(si, d_table[si])][1] + cost[(si, d)][1]
                v2 = V - cost[(si, d_table[si])][2] + cost[(si, d)][2]
                m = max(g2, v2)
                if best_move is None or m < best_move[0]:
                    best_move = (m, si, d)
        if best_move is None or best_move[0] >= max(G, V) - 1.0:
            break
        d_table[best_move[1]] = best_move[2]
    lseg_table = [cost[(si, d_table[si])][0] for si in range(NSLOT_A)]
    d_table = [16] + d_table
    lseg_table = [[DUMMY_LSEG] * len(CHUNK_NXG)] + lseg_table
    # stream layout: per slot, per chunk: cn = nxg * lseg
    chunks = []
    o16 = ow = 0
    for si in range(SLOTS):
        d = d_table[si]
        nxgs = D_CHUNKS[d]
        for cidx, nxg in enumerate(nxgs):
            L = lseg_table[si][cidx]
            cn = nxg * L
            chunks.append(dict(si=si, cidx=cidx, d=d, L=L, cn=cn,
                               xoff=sum(nxgs[:cidx]), nxg=nxg,
                               o16=o16, ow=ow))
            o16 += cn // 16
            ow += cn * d
    tot16, totw = o16, ow
    maxcn = max(ch["cn"] for ch in chunks)
    maxels = max(ch["cn"] * ch["d"] for ch in chunks)

    # pass B: build per-core idx blobs [128, tot16] + w blobs [64, totw]
    core_idx = []
    core_w = []
    for ci, (var, angles) in enumerate(CORE_SPECS):
        idx_blob = np.zeros((128, tot16), np.int16)
        w_blob = np.zeros((64, totw), np.float32)
        for si in range(SLOTS):
            d = d_table[si]
            if si == 0 or si - 1 >= len(core_order[ci]):
                continue
            k = core_order[ci][si - 1]
            a = _angle_tiles(k, var, d)
            lsegs = lseg_table[si]
            sch = [c for c in chunks if c["si"] == si]
            nxgs = D_CHUNKS[d]
            edges = np.cumsum([0] + nxgs)
            # per-(bin,group) stream position of tile t:
            #   spos = chunk.o16*16 + (bin-lo)*L + t   (per-group stream)
            ub, ut = a["ubin2"], a["utile2"]
            # grp/t in o2 order: pos within bin
            starts = np.searchsorted(ub, np.arange(S + 1))
            pos = np.arange(len(ub)) - starts[ub]
            grp = pos % NGROUP
            tt = pos // NGROUP
            cid = np.searchsorted(edges, ub, side="right") - 1
            L_arr = np.array([lsegs[c] for c in range(len(nxgs))])
            off_arr = np.array([sch[c]["o16"] * 16 for c in range(len(nxgs))])
            assert np.all(tt < L_arr[cid]), (ci, si, k)
            spos = off_arr[cid] + (ub - edges[cid]) * L_arr[cid] + tt
            # scatter idx values: stream for group g wrapped into
            # partitions 16g..16g+15: idx[16g + (p%16), p//16] = val
            # default padding: repeat previous valid idx (avoid addr jumps)
            sv = np.zeros((NGROUP, tot16 * 16), np.int16)
            filled = np.zeros((NGROUP, tot16 * 16), bool)
            sv[grp, spos] = ut
            filled[grp, spos] = True
            # forward-fill padding within this slot's range
            lo16, hi16 = sch[0]["o16"] * 16, (sch[-1]["o16"] + sch[-1]["cn"] // 16) * 16
            for g in range(NGROUP):
                seg = sv[g, lo16:hi16]
                fil = filled[g, lo16:hi16]
                idxs = np.where(fil, np.arange(len(seg)), 0)
                np.maximum.accumulate(idxs, out=idxs)
                sv[g, lo16:hi16] = seg[idxs]
            # wrap into idx_blob
            for g in range(NGROUP):
                st = sv[g, lo16:hi16]
                wrap = st.reshape(-1, 16).T
                idx_blob[16 * g:16 * g + 16, lo16 // 16:hi16 // 16] = wrap
            # weights: per tap: blob row = 8*grp_tap + jc, col = spos*d + e
            tap_key = a["key"]
            urow = np.searchsorted(a["uk"], tap_key)
            tap_grp = a["grp_u"][urow]
            tap_t = a["t_u"][urow]
            tap_bin = a["bins"]
            tap_cid = np.searchsorted(edges, tap_bin, side="right") - 1
            tap_spos = (off_arr[tap_cid] + (tap_bin - edges[tap_cid])
                        * L_arr[tap_cid] + tap_t)
            wrow = 8 * tap_grp + a["jc"]
            ow_arr = np.array([sch[c]["ow"] for c in range(len(nxgs))])
            o16_arr = np.array([sch[c]["o16"] * 16 for c in range(len(nxgs))])
            wcol = ow_arr[tap_cid] + (tap_spos - o16_arr[tap_cid]) * d + a["e"]
            np.add.at(w_blob, (wrow, wcol), a["w"])
        core_idx.append(idx_blob)
        core_w.append(w_blob.astype(BF16))

    sel = np.zeros((128, 2), np.float32)
    for p in range(128):
        sel[p, p % 2] = 1.0
    plan = dict(d_table=d_table, lseg_table=lseg_table, chunks=chunks,
                tot16=tot16, totw=totw, maxcn=maxcn, maxels=maxels,
                core_idx=core_idx, core_w=core_w, sel=sel,
                core_order=core_order)
    _PLAN_CACHE["plan"] = plan
    return plan


def _build_slab(image, var):
    """[128, NELS] bf16: partition p=(g,jc,b): cols ≡ jc mod 8 of variant
    frame, batch b; element idx = xt*ROWS + row."""
    img = np.asarray(image, np.float32)[:, 0]
    # padded image: 256 -> 363
    pad_total = S - IMG_SIZE
    pb = pad_total // 2
    pimg = np.zeros((BATCH, S, S), np.float32)
    pimg[:, pb:pb + IMG_SIZE, pb:pb + IMG_SIZE] = img
    Yg, Xg = np.meshgrid(np.arange(S), np.arange(S), indexing="ij")
    r, c = _variant_rc(var, Yg, Xg)
    frame = np.zeros((BATCH, ROWS, NCOL), np.float32)
    frame[:, r, c] = pimg
    # slice: [16, XT, ROWS]
    slab16 = np.zeros((16, NELS), np.float32)
    for jc in range(8):
        cols = frame[:, :, jc::8]              # [B, ROWS, XT]
        sl = np.transpose(cols, (0, 2, 1)).reshape(BATCH, -1)  # xt-major rows
        for b in range(BATCH):
            slab16[2 * jc + b] = sl[b]
    return np.tile(slab16, (8, 1)).astype(BF16)


_PROG_CACHE = {}


def _build_program(plan):
    if "prog" in _PROG_CACHE:
        return _PROG_CACHE["prog"]
    import concourse.bass as bass
    import concourse.mybir as mybir
    from concourse import library_config

    chunks = plan["chunks"]
    maxcn = plan["maxcn"]
    maxels = plan["maxels"]

    nc = bass.Bass()
    slab_d = nc.declare_dram_parameter("slab", [128, NELS],
                                       mybir.dt.bfloat16, isOutput=False)
    idx_d = nc.declare_dram_parameter("idx", [128, plan["tot16"]],
                                      mybir.dt.int16, isOutput=False)
    w_d = nc.declare_dram_parameter("w", [64, plan["totw"]],
                                    mybir.dt.bfloat16, isOutput=False)
    sel_d = nc.declare_dram_parameter("sel", [128, 2], mybir.dt.float32,
                                      isOutput=False)
    out_d = nc.declare_dram_parameter("out", [SLOTS, 2, SXPAD],
                                      mybir.dt.float32, isOutput=True)

    ctx = ExitStack()
    with ctx:
        slab_t = ctx.enter_context(nc.sbuf_tensor([128, NELS], mybir.dt.bfloat16))
        idx_ts = [ctx.enter_context(nc.sbuf_tensor(f"idx{i}", [128, maxcn // 16], mybir.dt.int16)) for i in range(4)]
        w_ts = [ctx.enter_context(nc.sbuf_tensor(f"w{i}", [128, maxels], mybir.dt.bfloat16)) for i in range(4)]
        g_ts = [ctx.enter_context(nc.sbuf_tensor(f"g{i}", [128, maxels], mybir.dt.bfloat16)) for i in range(2)]
        p_t = ctx.enter_context(nc.sbuf_tensor([128, maxels], mybir.dt.bfloat16))
        r_ts = [ctx.enter_context(nc.sbuf_tensor(f"r{i}", [128, SXPAD], mybir.dt.float32)) for i in range(2)]
        sel_t = ctx.enter_context(nc.sbuf_tensor([128, 2], mybir.dt.float32))
        vscr_t = ctx.enter_context(nc.sbuf_tensor([128, 2], mybir.dt.float32))
        sino_ts = [ctx.enter_context(nc.sbuf_tensor(f"sino{i}", [2, SXPAD], mybir.dt.float32)) for i in range(2)]
        psum_ts = [ctx.enter_context(nc.psum_tensor(f"ps{i}", [2, SXPAD], mybir.dt.float32)) for i in range(2)]
        s_in = ctx.enter_context(nc.semaphore("s_in"))
        s_dma = ctx.enter_context(nc.semaphore("s_dma"))
        s_g = ctx.enter_context(nc.semaphore("s_g"))
        s_v = ctx.enter_context(nc.semaphore("s_v"))
        s_mm = ctx.enter_context(nc.semaphore("s_mm"))
        s_cp = ctx.enter_context(nc.semaphore("s_cp"))
        s_od = ctx.enter_context(nc.semaphore("s_od"))
        block = ctx.enter_context(nc.Block())

        slot_end = [0] * SLOTS
        for n, ch in enumerate(chunks):
            slot_end[ch["si"]] = n + 1

        @block.sync
        def _(sync):
            sync.dma_start(out=sel_t[:], in_=sel_d[:]).then_inc(s_in, 16)
            sync.dma_start(out=slab_t[:], in_=slab_d[:]).then_inc(s_in, 16)
            for n, ch in enumerate(chunks):
                # 4-deep prefetch: buffer n%4 was last used by chunk n-4
                # (idx read by gather n-4, w read by vector n-4); the deep
                # pipeline also gives cold-start DMAs time to actually land
                # (completion semaphores fire early).
                if n > 3:
                    sync.wait_ge(s_g, n - 3)
                    sync.wait_ge(s_v, n - 3)
                sync.dma_start(
                    out=idx_ts[n % 4][:, :ch["cn"] // 16],
                    in_=idx_d[:, ch["o16"]:ch["o16"] + ch["cn"] // 16],
                ).then_inc(s_dma, 16)
                cnd = ch["cn"] * ch["d"]
                wsrc = (w_d[:, ch["ow"]:ch["ow"] + cnd]
                        .unsqueeze(1).broadcast_to([64, 2, cnd]))
                sync.dma_start(out=w_ts[n % 4][:, :cnd], in_=wsrc).then_inc(s_dma, 16)

        @block.gpsimd
        def _(g):
            g.load_library(library_config.ap_gather)
            g.wait_ge(s_in, 32)
            g.wait_ge(s_dma, 32)
            # warmup: amortize ext-isa first-call cost + preamble margin
            ch0 = chunks[0]
            d0 = ch0["d"]
            for _ in range(2):
                g.ap_gather(
                    g_ts[1][:, :ch0["cn"] * d0].rearrange("p (n d) -> p n d", d=d0),
                    slab_t[:].rearrange("p (n d) -> p n d", d=d0),
                    idx_ts[0][:, :ch0["cn"] // 16],
                    channels=128, num_elems=NELS // d0, d=d0, num_idxs=ch0["cn"],
                )
            for n, ch in enumerate(chunks):
                d = ch["d"]
                g.wait_ge(s_dma, 32 * (n + 1))
                if n > 1:
                    g.wait_ge(s_v, n - 1)
                g.ap_gather(
                    g_ts[n % 2][:, :ch["cn"] * d].rearrange("p (n d) -> p n d", d=d),
                    slab_t[:].rearrange("p (n d) -> p n d", d=d),
                    idx_ts[n % 4][:, :ch["cn"] // 16],
                    channels=128, num_elems=NELS // d, d=d, num_idxs=ch["cn"],
                ).then_inc(s_g, 1)

        @block.vector
        def _(v):
            for n, ch in enumerate(chunks):
                v.wait_ge(s_g, n + 1)
                if ch["cidx"] == 0 and ch["si"] > 1:
                    v.wait_ge(s_mm, ch["si"] - 1)
                cnd = ch["cn"] * ch["d"]
                v.tensor_mul(p_t[:, :cnd], g_ts[n % 2][:, :cnd],
                             w_ts[n % 4][:, :cnd])
                rdst = r_ts[ch["si"] % 2]
                v.tensor_reduce(
                    out=rdst[:, ch["xoff"]:ch["xoff"] + ch["nxg"]],
                    in_=p_t[:, :cnd].rearrange(
                        "p (x l) -> p x l", l=ch["L"] * ch["d"]),
                    axis=mybir.AxisListType.X,
                    op=mybir.AluOpType.add,
                )
                v.tensor_copy(vscr_t[:, :1],
                              rdst[:, ch["xoff"]:ch["xoff"] + 1]).then_inc(s_v, 1)

        @block.tensor
        def _(t):
            t.wait_ge(s_in, 32)
            for si in range(SLOTS):
                t.wait_ge(s_v, slot_end[si])
                if si > 1:
                    t.wait_ge(s_cp, si - 1)
                t.matmul(psum_ts[si % 2][:], sel_t[:], r_ts[si % 2][:],
                         start=True, stop=True).then_inc(s_mm, 1)

        @block.scalar
        def _(sc):
            for si in range(SLOTS):
                sc.wait_ge(s_mm, si + 1)
                if si > 1:
                    sc.wait_ge(s_od, 16 * (si - 1))  # sino buf freed by DMA
                sc.copy(sino_ts[si % 2][:], psum_ts[si % 2][:]).then_inc(s_cp, 1)
                sc.dma_start(out=out_d[si], in_=sino_ts[si % 2][:]
                             ).then_inc(s_od, 16)
            sc.wait_ge(s_od, 16 * SLOTS)

    import concourse.mybir as mybir2
    mybir2.codegen_inst_isa_subclasses(nc)
    _PROG_CACHE["prog"] = nc
    return nc


def kernel(image):
    image = np.asarray(image, np.float32)
    assert image.shape == (BATCH, 1, IMG_SIZE, IMG_SIZE)
    plan = _get_plan()
    nc = _build_program(plan)

    from concourse.bass_utils import run_bass_kernel_spmd

    in_maps = []
    for ci, (var, angles) in enumerate(CORE_SPECS):
        in_maps.append({
            "slab": _build_slab(image, var),
            "idx": plan["core_idx"][ci],
            "w": plan["core_w"][ci],
            "sel": plan["sel"],
        })

    trace = bool(os.environ.get("RADON_TRACE"))
    if trace:
        _install_profhook()
    res = run_bass_kernel_spmd(nc, in_maps, list(range(8)), trace=trace)
    if trace:
        kernel.last_exec_time_ns = res.exec_time_ns

    sino = np.zeros((BATCH, 1, S, N_ANGLES), np.float32)
    for ci in range(8):
        o = res.results[ci]["out"]  # [SLOTS, 2, SXPAD]; slot 0 is dummy
        for si, k in enumerate(plan["core_order"][ci]):
            sino[:, 0, :, k] = o[si + 1, :, :S]
    return sino


def _install_profhook():
    import types
    if "antenv.axon_hooks" in sys.modules:
        return
    try:
        from trn_agent_boot.trn_boot import _ntff_profile_via_ctypes
        hook = _ntff_profile_via_ctypes("/opt/axon/libaxon_pjrt.so")
    except Exception:
        hook = None
    mod = types.ModuleType("antenv.axon_hooks")
    mod._hook = hook
    mod.set_axon_ntff_profile_hook = lambda h: setattr(mod, "_hook", h)
    mod.get_axon_ntff_profile_hook = lambda: mod._hook
    sys.modules["antenv.axon_hooks"] = mod
    import antenv
    antenv.axon_hooks = mod


if __name__ == "__main__":
    img = np.load("/tmp/ref_image.npy")
    out = kernel(image=img)
    exp = np.load("/tmp/ref_expected.npy")
    err = np.linalg.norm(out - exp) / np.linalg.norm(exp)
    print("kernel rel err:", err)


# revision 44
# speedup vs baseline: 1.0284x; 1.0107x over previous
"""Radon transform (bilinear grid-sample + row-sum) on 8 TRN2 NeuronCores.

Tile-gather design: each angle's sample lines are walked in a frame variant
(identity / transpose / +-diagonal shear) chosen so the line advances < 0.59
columns per row.  The frame is stored column-sliced across each 16-partition
GPSIMD group (partition = (col mod 8, batch)); one ap_gather index fetches a
d x 8 pixel tile (d rows x 8 adjacent columns x 2 batches) in one shot, so
the per-index RD_CMD latency (the kernel bottleneck) is amortized over a
whole 2-D tile instead of a single bilinear tap pair.  All 8 groups hold
identical slabs; a bin's tiles are dealt round-robin to groups.  DVE applies
precomputed weights + segment-reduce per sinogram bin; a ones-ish matmul
sums partitions; per-slot DMA returns [2, 368] sinogram columns.

All gather indices / weights are input-independent and precomputed on host.
"""
import math
import os
import sys
from contextlib import ExitStack

import numpy as np

sys.path.insert(0, "/opt/trn_rl_repo")

import ml_dtypes  # noqa: E402

BF16 = ml_dtypes.bfloat16

# ─── geometry constants (hardcoded for 256x256, 180 angles, batch 2) ───
N_ANGLES = 180
IMG_SIZE = 256
BATCH = 2
S = int(math.ceil(math.sqrt(2.0) * IMG_SIZE))  # 363

ROWS = 384            # slab rows (multiple of 32)
XT = 92               # col-tiles per partition slice
NELS = XT * ROWS      # 35328 elements per partition
NCOL = XT * 8         # 736 layout columns
SXPAD = 368           # padded bin count
SLOTS = 24            # slot 0 is a dummy pipeline-priming slot
DUMMY_LSEG = 3
NGROUP = 8
D_CHOICES = (8, 16, 24, 32, 48)
# per-d bin chunking: nxg multiples of 16 (index wrap), sized so
# nxg*lseg*d stays under the SBUF chunk-volume cap even at large lseg
D_CHUNKS = {
    8: [64, 64, 64, 64, 64, 48],
    16: [64, 64, 64, 64, 64, 48],
    24: [64, 64, 64, 64, 64, 48],
    32: [64, 64, 64, 64, 64, 48],
    48: [64, 64, 64, 64, 64, 48],
}
CHUNK_NXG = D_CHUNKS[16]  # dummy-slot scheme
MAXVOL = 8192
# hardware-measured per-index gather cost (cycles) by block depth d
GATHER_CYC = {2: 33.0, 8: 40.0, 16: 39.5, 24: 56.8, 32: 62.3, 48: 76.0}
DVE_NS_PER_EL = 1.33     # mult (2.2 el/ns) + segment reduce (1.13 el/ns)
CHUNK_OVERHEAD_NS = 0     # per-chunk gather dispatch/idx-load/sync cost

# angle classes: (variant, angle list). variant row/col maps:
#   v0: row=Y+1, col=X+1            (|tan| <= tan22.5, theta near 0/180)
#   v1: row=X+1, col=Y+1            (theta near 90)
#   v2a: row=Y+1, col=X-Y+368       (22.5 < th <= 45)
#   v2b: row=X+1, col=X-Y+368       (45 < th < 67.5)
#   v3a: row=X+1, col=X+Y+2         (112.5 < th <= 135)
#   v3b: row=Y+1, col=X+Y+2         (135 < th < 157.5)
CORE_SPECS = [
    ("v0", list(range(0, 23))),
    ("v0", list(range(158, 180))),
    ("v2a", list(range(23, 46))),
    ("v2b", list(range(46, 68))),
    ("v1", list(range(68, 91))),
    ("v1", list(range(91, 113))),
    ("v3a", list(range(113, 136))),
    ("v3b", list(range(136, 158))),
]


def _angle_tables(k):
    theta = np.float32(k) * np.float32(np.pi / N_ANGLES)
    c = np.cos(theta, dtype=np.float32)
    s = np.sin(theta, dtype=np.float32)
    lin = np.linspace(-1.0, 1.0, S, dtype=np.float32)
    x = lin[None, :]
    y = lin[:, None]
    gx = c * x + s * y
    gy = -s * x + c * y
    ix = ((gx + np.float32(1.0)) * np.float32(0.5) * np.float32(S - 1)).astype(np.float32)
    iy = ((gy + np.float32(1.0)) * np.float32(0.5) * np.float32(S - 1)).astype(np.float32)
    x0 = np.floor(ix)
    y0 = np.floor(iy)
    wx = ix - x0
    wy = iy - y0
    return y0.astype(np.int64), x0.astype(np.int64), wx, wy


def _variant_rc(var, Y, X):
    if var == "v0":
        return Y + 1, X + 1
    if var == "v1":
        return X + 1, Y + 1
    if var == "v2a":
        return Y + 1, X - Y + 368
    if var == "v2b":
        return X + 1, X - Y + 368
    if var == "v3a":
        return X + 1, X + Y + 2
    return Y + 1, X + Y + 2  # v3b


def _angle_taps(k, var):
    """Flat arrays over valid taps: bin, samp, row, col, w (f32)."""
    y0, x0, wx, wy = _angle_tables(k)
    kk = np.broadcast_to(np.arange(S, dtype=np.int64)[:, None], (S, S))
    bb = np.broadcast_to(np.arange(S, dtype=np.int64)[None, :], (S, S))
    outs = []
    for dy in (0, 1):
        for dx in (0, 1):
            Y = y0 + dy
            X = x0 + dx
            w = (wy if dy else (1.0 - wy)) * (wx if dx else (1.0 - wx))
            v = (Y >= 0) & (Y < S) & (X >= 0) & (X < S)
            r, c = _variant_rc(var, Y, X)
            outs.append((bb[v], kk[v], r[v], c[v], w[v].astype(np.float32)))
    bins = np.concatenate([o[0] for o in outs])
    samp = np.concatenate([o[1] for o in outs])
    row = np.concatenate([o[2] for o in outs])
    col = np.concatenate([o[3] for o in outs])
    w = np.concatenate([o[4] for o in outs])
    return bins, samp, row, col, w


def _angle_tiles(k, var, d):
    """Per-bin ordered unique tiles + per-tap (group, slot, channel, elem).

    Returns dict with:
      lsegs[6]      per-chunk max segment length (ceil(ntiles/8))
      (after _plan pass B) idx/w scatter info
    """
    bins, samp, row, col, w = _angle_taps(k, var)
    rb = row // d
    xt = col // 8
    jc = col % 8
    e = row % d
    nrb = ROWS // d
    tile = xt * nrb + rb                       # block index in slab (< 4416)
    key = bins * (XT * nrb) + tile
    order = np.lexsort((samp, key))
    key_s = key[order]
    samp_s = samp[order]
    uk, first = np.unique(key_s, return_index=True)
    firstk = samp_s[first]                     # first sample touching tile
    ubin = uk // (XT * nrb)
    utile = uk % (XT * nrb)
    # order tiles within bin by first-sample
    o2 = np.lexsort((firstk, ubin))
    ubin2 = ubin[o2]
    utile2 = utile[o2]
    starts = np.searchsorted(ubin2, np.arange(S + 1))
    pos = np.arange(len(ubin2)) - starts[ubin2]
    ntiles = np.diff(starts)                   # tiles per bin
    # invert o2: for each unique-key row, its (group, t)
    grp_u = np.empty(len(uk), np.int64)
    t_u = np.empty(len(uk), np.int64)
    grp_u[o2] = pos % NGROUP
    t_u[o2] = pos // NGROUP
    # per-chunk lsegs
    edges = np.cumsum([0] + CHUNK_NXG)
    lsegs = []
    for ci in range(len(CHUNK_NXG)):
        lo, hi = edges[ci], min(edges[ci + 1], S)
        nt = ntiles[lo:hi] if hi > lo else np.zeros(1, np.int64)
        lsegs.append(int(np.ceil(nt.max() / NGROUP)) if len(nt) else 1)
    return dict(lsegs=[max(l, 1) for l in lsegs], uk=uk, ubin2=ubin2,
                utile2=utile2, grp_u=grp_u, t_u=t_u, key=key, jc=jc, e=e,
                w=w, bins=bins)


def _gather_cyc(nidx, d):
    return nidx * GATHER_CYC[d]


_PLAN_CACHE = {}


def _get_plan():
    if "plan" in _PLAN_CACHE:
        return _PLAN_CACHE["plan"]
    # pass A: per (core, angle, d): lsegs + cost -> per-core angle order,
    # global d_table / lseg_table.  All d are multiples of 8, so tile sets
    # for every d derive from one unique pass at d=8.
    info = {}
    for ci, (var, angles) in enumerate(CORE_SPECS):
        for k in angles:
            bins, samp, row, col, w = _angle_taps(k, var)
            key8 = (bins * XT + col // 8) * 48 + row // 8
            u8 = np.unique(key8)
            ubin8 = u8 // (48 * XT)
            uxt8 = (u8 // 48) % XT
            urb8 = u8 % 48
            per_d = {}
            for d in D_CHOICES:
                m = d // 8
                kd = (ubin8 * XT + uxt8) * 48 + urb8 // m
                ud = np.unique(kd)
                nt = np.bincount(ud // (48 * XT), minlength=S)
                nxgs = D_CHUNKS[d]
                edges = np.cumsum([0] + nxgs)
                lsegs = []
                for cix in range(len(nxgs)):
                    lo, hi = edges[cix], min(edges[cix + 1], S)
                    mx = nt[lo:hi].max() if hi > lo else 0
                    lsegs.append(max(int(np.ceil(mx / NGROUP)), 1))
                nidx = sum(nxg * l for nxg, l in zip(nxgs, lsegs))
                gat_ns = _gather_cyc(nidx, d) / 1.2
                dve_ns = nidx * d * DVE_NS_PER_EL
                per_d[d] = (lsegs, max(gat_ns, dve_ns), gat_ns, dve_ns)
            info[k] = per_d
    # per-core angle order: hardest first (by best-d cost)
    core_order = []
    for ci, (var, angles) in enumerate(CORE_SPECS):
        best = {k: min(v[d][1] for d in D_CHOICES) for k, v in
                ((k, info[k]) for k in angles)}
        core_order.append(sorted(angles, key=lambda k: -best[k]))

    # per-slot, per-d: cross-core max lsegs -> (gather_ns, dve_ns)
    def slot_cost(si, d):
        ls = [1] * len(D_CHUNKS[d])
        for ci in range(8):
            if si < len(core_order[ci]):
                al = info[core_order[ci][si]][d][0]
                ls = [max(a, b) for a, b in zip(ls, al)]
        nidx = sum(nxg * l for nxg, l in zip(D_CHUNKS[d], ls))
        gat = _gather_cyc(nidx, d) / 1.2 + CHUNK_OVERHEAD_NS * len(D_CHUNKS[d])
        return ls, gat, nidx * d * DVE_NS_PER_EL

    NSLOT_A = SLOTS - 1   # angle slots (slot 0 is the dummy priming slot)
    cost = {}
    for si in range(NSLOT_A):
        for d in D_CHOICES:
            ls, gat, dve = slot_cost(si, d)
            # cap per-chunk gathered volume so 4-deep DMA buffers fit SBUF
            if max(nxg * l * d for nxg, l in zip(D_CHUNKS[d], ls)) > MAXVOL:
                gat = dve = 1e18
            cost[(si, d)] = (ls, gat, dve)
    # choose d per slot minimizing max(total gather, total dve)
    d_table = [min(D_CHOICES, key=lambda d: cost[(si, d)][1])
               for si in range(NSLOT_A)]
    while True:
        G = sum(cost[(si, d_table[si])][1] for si in range(NSLOT_A))
        V = sum(cost[(si, d_table[si])][2] for si in range(NSLOT_A))
        best_move = None
        for si in range(NSLOT_A):
            for d in D_CHOICES:
                if d == d_table[si]:
                    continue
                g2 = G - cost[(si, d_table[si])][1] + cost[(si, d)][1]
                v2 = V - cost[(si, d_table[si])][2] + cost[(si, d)][2]
                m = max(g2, v2)
                if best_move is None or m < best_move[0]:
                    best_move = (m, si, d)
        if best_move is None or best_move[0] >= max(G, V) - 1.0:
            break
        d_table[best_move[1]] = best_move[2]
    lseg_table = [cost[(si, d_table[si])][0] for si in range(NSLOT_A)]
    d_table = [16] + d_table
    lseg_table = [[DUMMY_LSEG] * len(CHUNK_NXG)] + lseg_table
    # stream layout: per slot, per chunk: cn = nxg * lseg
    chunks = []
    o16 = ow = 0
    for si in range(SLOTS):
        d = d_table[si]
        nxgs = D_CHUNKS[d]
        for cidx, nxg in enumerate(nxgs):
            L = lseg_table[si][cidx]
            cn = nxg * L
            chunks.append(dict(si=si, cidx=cidx, d=d, L=L, cn=cn,
                               xoff=sum(nxgs[:cidx]), nxg=nxg,
                               o16=o16, ow=ow))
            o16 += cn // 16
            ow += cn * d
    tot16, totw = o16, ow
    maxcn = max(ch["cn"] for ch in chunks)
    maxels = max(ch["cn"] * ch["d"] for ch in chunks)

    # pass B: build per-core idx blobs [128, tot16] + w blobs [64, totw]
    core_idx = []
    core_w = []
    for ci, (var, angles) in enumerate(CORE_SPECS):
        idx_blob = np.zeros((128, tot16), np.int16)
        w_blob = np.zeros((64, totw), np.float32)
        for si in range(SLOTS):
            d = d_table[si]
            if si == 0 or si - 1 >= len(core_order[ci]):
                continue
            k = core_order[ci][si - 1]
            a = _angle_tiles(k, var, d)
            lsegs = lseg_table[si]
            sch = [c for c in chunks if c["si"] == si]
            nxgs = D_CHUNKS[d]
            edges = np.cumsum([0] + nxgs)
            # per-(bin,group) stream position of tile t:
            #   spos = chunk.o16*16 + (bin-lo)*L + t   (per-group stream)
            ub, ut = a["ubin2"], a["utile2"]
            # grp/t in o2 order: pos within bin
            starts = np.searchsorted(ub, np.arange(S + 1))
            pos = np.arange(len(ub)) - starts[ub]
            grp = pos % NGROUP
            tt = pos // NGROUP
            cid = np.searchsorted(edges, ub, side="right") - 1
            L_arr = np.array([lsegs[c] for c in range(len(nxgs))])
            off_arr = np.array([sch[c]["o16"] * 16 for c in range(len(nxgs))])
            assert np.all(tt < L_arr[cid]), (ci, si, k)
            spos = off_arr[cid] + (ub - edges[cid]) * L_arr[cid] + tt
            # scatter idx values: stream for group g wrapped into
            # partitions 16g..16g+15: idx[16g + (p%16), p//16] = val
            # default padding: repeat previous valid idx (avoid addr jumps)
            sv = np.zeros((NGROUP, tot16 * 16), np.int16)
            filled = np.zeros((NGROUP, tot16 * 16), bool)
            sv[grp, spos] = ut
            filled[grp, spos] = True
            # forward-fill padding within this slot's range
            lo16, hi16 = sch[0]["o16"] * 16, (sch[-1]["o16"] + sch[-1]["cn"] // 16) * 16
            for g in range(NGROUP):
                seg = sv[g, lo16:hi16]
                fil = filled[g, lo16:hi16]
                idxs = np.where(fil, np.arange(len(seg)), 0)
                np.maximum.accumulate(idxs, out=idxs)
                sv[g, lo16:hi16] = seg[idxs]
            # wrap into idx_blob
            for g in range(NGROUP):
                st = sv[g, lo16:hi16]
                wrap = st.reshape(-1, 16).T
                idx_blob[16 * g:16 * g + 16, lo16 // 16:hi16 // 16] = wrap
            # weights: per tap: blob row = 8*grp_tap + jc, col = spos*d + e
            tap_key = a["key"]
            urow = np.searchsorted(a["uk"], tap_key)
            tap_grp = a["grp_u"][urow]
            tap_t = a["t_u"][urow]
            tap_bin = a["bins"]
            tap_cid = np.searchsorted(edges, tap_bin, side="right") - 1
            tap_spos = (off_arr[tap_cid] + (tap_bin - edges[tap_cid])
                        * L_arr[tap_cid] + tap_t)
            wrow = 8 * tap_grp + a["jc"]
            ow_arr = np.array([sch[c]["ow"] for c in range(len(nxgs))])
            o16_arr = np.array([sch[c]["o16"] * 16 for c in range(len(nxgs))])
            wcol = ow_arr[tap_cid] + (tap_spos - o16_arr[tap_cid]) * d + a["e"]
            np.add.at(w_blob, (wrow, wcol), a["w"])
        core_idx.append(idx_blob)
        core_w.append(w_blob.astype(BF16))

    sel = np.zeros((128, 2), np.float32)
    for p in range(128):
        sel[p, p % 2] = 1.0
    plan = dict(d_table=d_table, lseg_table=lseg_table, chunks=chunks,
                tot16=tot16, totw=totw, maxcn=maxcn, maxels=maxels,
                core_idx=core_idx, core_w=core_w, sel=sel,
                core_order=core_order)
    _PLAN_CACHE["plan"] = plan
    return plan


def _build_slab(image, var):
    """[128, NELS] bf16: partition p=(g,jc,b): cols ≡ jc mod 8 of variant
    frame, batch b; element idx = xt*ROWS + row."""
    img = np.asarray(image, np.float32)[:, 0]
    # padded image: 256 -> 363
    pad_total = S - IMG_SIZE
    pb = pad_total // 2
    pimg = np.zeros((BATCH, S, S), np.float32)
    pimg[:, pb:pb + IMG_SIZE, pb:pb + IMG_SIZE] = img
    Yg, Xg = np.meshgrid(np.arange(S), np.arange(S), indexing="ij")
    r, c = _variant_rc(var, Yg, Xg)
    frame = np.zeros((BATCH, ROWS, NCOL), np.float32)
    frame[:, r, c] = pimg
    # slice: [16, XT, ROWS]
    slab16 = np.zeros((16, NELS), np.float32)
    for jc in range(8):
        cols = frame[:, :, jc::8]              # [B, ROWS, XT]
        sl = np.transpose(cols, (0, 2, 1)).reshape(BATCH, -1)  # xt-major rows
        for b in range(BATCH):
            slab16[2 * jc + b] = sl[b]
    return np.tile(slab16, (8, 1)).astype(BF16)


_PROG_CACHE = {}


def _build_program(plan):
    if "prog" in _PROG_CACHE:
        return _PROG_CACHE["prog"]
    import concourse.bass as bass
    import concourse.mybir as mybir
    from concourse import library_config

    chunks = plan["chunks"]
    maxcn = plan["maxcn"]
    maxels = plan["maxels"]

    nc = bass.Bass()
    slab_d = nc.declare_dram_parameter("slab", [128, NELS],
                                       mybir.dt.bfloat16, isOutput=False)
    idx_d = nc.declare_dram_parameter("idx", [128, plan["tot16"]],
                                      mybir.dt.int16, isOutput=False)
    w_d = nc.declare_dram_parameter("w", [64, plan["totw"]],
                                    mybir.dt.bfloat16, isOutput=False)
    sel_d = nc.declare_dram_parameter("sel", [128, 2], mybir.dt.float32,
                                      isOutput=False)
    out_d = nc.declare_dram_parameter("out", [SLOTS, 2, SXPAD],
                                      mybir.dt.float32, isOutput=True)

    ctx = ExitStack()
    with ctx:
        slab_t = ctx.enter_context(nc.sbuf_tensor([128, NELS], mybir.dt.bfloat16))
        idx_ts = [ctx.enter_context(nc.sbuf_tensor(f"idx{i}", [128, maxcn // 16], mybir.dt.int16)) for i in range(4)]
        w_ts = [ctx.enter_context(nc.sbuf_tensor(f"w{i}", [128, maxels], mybir.dt.bfloat16)) for i in range(4)]
        g_ts = [ctx.enter_context(nc.sbuf_tensor(f"g{i}", [128, maxels], mybir.dt.bfloat16)) for i in range(3)]
        p_t = ctx.enter_context(nc.sbuf_tensor([128, maxels], mybir.dt.bfloat16))
        r_ts = [ctx.enter_context(nc.sbuf_tensor(f"r{i}", [128, SXPAD], mybir.dt.float32)) for i in range(2)]
        sel_t = ctx.enter_context(nc.sbuf_tensor([128, 2], mybir.dt.float32))
        vscr_t = ctx.enter_context(nc.sbuf_tensor([128, 2], mybir.dt.float32))
        sino_ts = [ctx.enter_context(nc.sbuf_tensor(f"sino{i}", [2, SXPAD], mybir.dt.float32)) for i in range(2)]
        psum_ts = [ctx.enter_context(nc.psum_tensor(f"ps{i}", [2, SXPAD], mybir.dt.float32)) for i in range(2)]
        s_in = ctx.enter_context(nc.semaphore("s_in"))
        s_dma = ctx.enter_context(nc.semaphore("s_dma"))
        s_g = ctx.enter_context(nc.semaphore("s_g"))
        s_v = ctx.enter_context(nc.semaphore("s_v"))
        s_mm = ctx.enter_context(nc.semaphore("s_mm"))
        s_cp = ctx.enter_context(nc.semaphore("s_cp"))
        s_od = ctx.enter_context(nc.semaphore("s_od"))
        block = ctx.enter_context(nc.Block())

        slot_end = [0] * SLOTS
        for n, ch in enumerate(chunks):
            slot_end[ch["si"]] = n + 1

        @block.sync
        def _(sync):
            sync.dma_start(out=sel_t[:], in_=sel_d[:]).then_inc(s_in, 16)
            sync.dma_start(out=slab_t[:], in_=slab_d[:]).then_inc(s_in, 16)
            for n, ch in enumerate(chunks):
                # 4-deep prefetch: buffer n%4 was last used by chunk n-4
                # (idx read by gather n-4, w read by vector n-4); the deep
                # pipeline also gives cold-start DMAs time to actually land
                # (completion semaphores fire early).
                if n > 3:
                    sync.wait_ge(s_g, n - 3)
                    sync.wait_ge(s_v, n - 3)
                sync.dma_start(
                    out=idx_ts[n % 4][:, :ch["cn"] // 16],
                    in_=idx_d[:, ch["o16"]:ch["o16"] + ch["cn"] // 16],
                ).then_inc(s_dma, 16)
                cnd = ch["cn"] * ch["d"]
                wsrc = (w_d[:, ch["ow"]:ch["ow"] + cnd]
                        .unsqueeze(1).broadcast_to([64, 2, cnd]))
                sync.dma_start(out=w_ts[n % 4][:, :cnd], in_=wsrc).then_inc(s_dma, 16)

        @block.gpsimd
        def _(g):
            g.load_library(library_config.ap_gather)
            g.wait_ge(s_in, 32)
            g.wait_ge(s_dma, 32)
            # warmup: amortize ext-isa first-call cost + preamble margin
            ch0 = chunks[0]
            d0 = ch0["d"]
            for _ in range(2):
                g.ap_gather(
                    g_ts[2][:, :ch0["cn"] * d0].rearrange("p (n d) -> p n d", d=d0),
                    slab_t[:].rearrange("p (n d) -> p n d", d=d0),
                    idx_ts[0][:, :ch0["cn"] // 16],
                    channels=128, num_elems=NELS // d0, d=d0, num_idxs=ch0["cn"],
                )
            for n, ch in enumerate(chunks):
                d = ch["d"]
                g.wait_ge(s_dma, 32 * (n + 1))
                if n > 2:
                    g.wait_ge(s_v, n - 2)  # g_ts[n%3] consumed by vector n-3
                g.ap_gather(
                    g_ts[n % 3][:, :ch["cn"] * d].rearrange("p (n d) -> p n d", d=d),
                    slab_t[:].rearrange("p (n d) -> p n d", d=d),
                    idx_ts[n % 4][:, :ch["cn"] // 16],
                    channels=128, num_elems=NELS // d, d=d, num_idxs=ch["cn"],
                ).then_inc(s_g, 1)

        @block.vector
        def _(v):
            for n, ch in enumerate(chunks):
                v.wait_ge(s_g, n + 1)
                if ch["cidx"] == 0 and ch["si"] > 1:
                    v.wait_ge(s_mm, ch["si"] - 1)
                cnd = ch["cn"] * ch["d"]
                v.tensor_mul(p_t[:, :cnd], g_ts[n % 3][:, :cnd],
                             w_ts[n % 4][:, :cnd])
                rdst = r_ts[ch["si"] % 2]
                v.tensor_reduce(
                    out=rdst[:, ch["xoff"]:ch["xoff"] + ch["nxg"]],
                    in_=p_t[:, :cnd].rearrange(
                        "p (x l) -> p x l", l=ch["L"] * ch["d"]),
                    axis=mybir.AxisListType.X,
                    op=mybir.AluOpType.add,
                )
                v.tensor_copy(vscr_t[:, :1],
                              rdst[:, ch["xoff"]:ch["xoff"] + 1]).then_inc(s_v, 1)

        @block.tensor
        def _(t):
            t.wait_ge(s_in, 32)
            for si in range(SLOTS):
                t.wait_ge(s_v, slot_end[si])
                if si > 1:
                    t.wait_ge(s_cp, si - 1)
                t.matmul(psum_ts[si % 2][:], sel_t[:], r_ts[si % 2][:],
                         start=True, stop=True).then_inc(s_mm, 1)

        @block.scalar
        def _(sc):
            for si in range(SLOTS):
                sc.wait_ge(s_mm, si + 1)
                if si > 1:
                    sc.wait_ge(s_od, 16 * (si - 1))  # sino buf freed by DMA
                sc.copy(sino_ts[si % 2][:], psum_ts[si % 2][:]).then_inc(s_cp, 1)
                sc.dma_start(out=out_d[si], in_=sino_ts[si % 2][:]
                             ).then_inc(s_od, 16)
            sc.wait_ge(s_od, 16 * SLOTS)

    import concourse.mybir as mybir2
    mybir2.codegen_inst_isa_subclasses(nc)
    _PROG_CACHE["prog"] = nc
    return nc


def kernel(image):
    image = np.asarray(image, np.float32)
    assert image.shape == (BATCH, 1, IMG_SIZE, IMG_SIZE)
    plan = _get_plan()
    nc = _build_program(plan)

    from concourse.bass_utils import run_bass_kernel_spmd

    in_maps = []
    for ci, (var, angles) in enumerate(CORE_SPECS):
        in_maps.append({
            "slab": _build_slab(image, var),
            "idx": plan["core_idx"][ci],
            "w": plan["core_w"][ci],
            "sel": plan["sel"],
        })

    trace = bool(os.environ.get("RADON_TRACE"))
    if trace:
        _install_profhook()
    res = run_bass_kernel_spmd(nc, in_maps, list(range(8)), trace=trace)
    if trace:
        kernel.last_exec_time_ns = res.exec_time_ns

    sino = np.zeros((BATCH, 1, S, N_ANGLES), np.float32)
    for ci in range(8):
        o = res.results[ci]["out"]  # [SLOTS, 2, SXPAD]; slot 0 is dummy
        for si, k in enumerate(plan["core_order"][ci]):
            sino[:, 0, :, k] = o[si + 1, :, :S]
    return sino


def _install_profhook():
    import types
    if "antenv.axon_hooks" in sys.modules:
        return
    try:
        from trn_agent_boot.trn_boot import _ntff_profile_via_ctypes
        hook = _ntff_profile_via_ctypes("/opt/axon/libaxon_pjrt.so")
    except Exception:
        hook = None
    mod = types.ModuleType("antenv.axon_hooks")
    mod._hook = hook
    mod.set_axon_ntff_profile_hook = lambda h: setattr(mod, "_hook", h)
    mod.get_axon_ntff_profile_hook = lambda: mod._hook
    sys.modules["antenv.axon_hooks"] = mod
    import antenv
    antenv.axon_hooks = mod


if __name__ == "__main__":
    img = np.load("/tmp/ref_image.npy")
    out = kernel(image=img)
    exp = np.load("/tmp/ref_expected.npy")
    err = np.linalg.norm(out - exp) / np.linalg.norm(exp)
    print("kernel rel err:", err)


# revision 45
# speedup vs baseline: 1.0320x; 1.0034x over previous
"""Radon transform (bilinear grid-sample + row-sum) on 8 TRN2 NeuronCores.

Tile-gather design: each angle's sample lines are walked in a frame variant
(identity / transpose / +-diagonal shear) chosen so the line advances < 0.59
columns per row.  The frame is stored column-sliced across each 16-partition
GPSIMD group (partition = (col mod 8, batch)); one ap_gather index fetches a
d x 8 pixel tile (d rows x 8 adjacent columns x 2 batches) in one shot, so
the per-index RD_CMD latency (the kernel bottleneck) is amortized over a
whole 2-D tile instead of a single bilinear tap pair.  All 8 groups hold
identical slabs; a bin's tiles are dealt round-robin to groups.  DVE applies
precomputed weights + segment-reduce per sinogram bin; a ones-ish matmul
sums partitions; per-slot DMA returns [2, 368] sinogram columns.

All gather indices / weights are input-independent and precomputed on host.
"""
import math
import os
import sys
from contextlib import ExitStack

import numpy as np

sys.path.insert(0, "/opt/trn_rl_repo")

import ml_dtypes  # noqa: E402

BF16 = ml_dtypes.bfloat16

# ─── geometry constants (hardcoded for 256x256, 180 angles, batch 2) ───
N_ANGLES = 180
IMG_SIZE = 256
BATCH = 2
S = int(math.ceil(math.sqrt(2.0) * IMG_SIZE))  # 363

ROWS = 384            # slab rows (multiple of 32)
XT = 92               # col-tiles per partition slice
NELS = XT * ROWS      # 35328 elements per partition
NCOL = XT * 8         # 736 layout columns
SXPAD = 368           # padded bin count
SLOTS = 24            # slot 0 is a dummy pipeline-priming slot
DUMMY_LSEG = 3
NGROUP = 8
D_CHOICES = (8, 16, 24, 32, 48)
# per-d bin chunking: nxg multiples of 16 (index wrap), sized so
# nxg*lseg*d stays under the SBUF chunk-volume cap even at large lseg
D_CHUNKS = {
    8: [64, 64, 64, 64, 64, 48],
    16: [64, 64, 64, 64, 64, 48],
    24: [64, 64, 64, 64, 64, 48],
    32: [64, 64, 64, 64, 64, 48],
    48: [64, 64, 64, 64, 64, 48],
}
CHUNK_NXG = D_CHUNKS[16]  # dummy-slot scheme
MAXVOL = 8192
# hardware-measured per-index gather cost (cycles) by block depth d
GATHER_CYC = {2: 33.0, 8: 40.0, 16: 39.5, 24: 56.8, 32: 62.3, 48: 76.0}
DVE_NS_PER_EL = 1.33     # mult (2.2 el/ns) + segment reduce (1.13 el/ns)
CHUNK_OVERHEAD_NS = 0     # per-chunk gather dispatch/idx-load/sync cost

# angle classes: (variant, angle list). variant row/col maps:
#   v0: row=Y+1, col=X+1            (|tan| <= tan22.5, theta near 0/180)
#   v1: row=X+1, col=Y+1            (theta near 90)
#   v2a: row=Y+1, col=X-Y+368       (22.5 < th <= 45)
#   v2b: row=X+1, col=X-Y+368       (45 < th < 67.5)
#   v3a: row=X+1, col=X+Y+2         (112.5 < th <= 135)
#   v3b: row=Y+1, col=X+Y+2         (135 < th < 157.5)
CORE_SPECS = [
    ("v0", list(range(0, 23))),
    ("v0", list(range(158, 180))),
    ("v2a", list(range(23, 46))),
    ("v2b", list(range(46, 68))),
    ("v1", list(range(68, 91))),
    ("v1", list(range(91, 113))),
    ("v3a", list(range(113, 136))),
    ("v3b", list(range(136, 158))),
]


def _angle_tables(k):
    theta = np.float32(k) * np.float32(np.pi / N_ANGLES)
    c = np.cos(theta, dtype=np.float32)
    s = np.sin(theta, dtype=np.float32)
    lin = np.linspace(-1.0, 1.0, S, dtype=np.float32)
    x = lin[None, :]
    y = lin[:, None]
    gx = c * x + s * y
    gy = -s * x + c * y
    ix = ((gx + np.float32(1.0)) * np.float32(0.5) * np.float32(S - 1)).astype(np.float32)
    iy = ((gy + np.float32(1.0)) * np.float32(0.5) * np.float32(S - 1)).astype(np.float32)
    x0 = np.floor(ix)
    y0 = np.floor(iy)
    wx = ix - x0
    wy = iy - y0
    return y0.astype(np.int64), x0.astype(np.int64), wx, wy


def _variant_rc(var, Y, X):
    if var == "v0":
        return Y + 1, X + 1
    if var == "v1":
        return X + 1, Y + 1
    if var == "v2a":
        return Y + 1, X - Y + 368
    if var == "v2b":
        return X + 1, X - Y + 368
    if var == "v3a":
        return X + 1, X + Y + 2
    return Y + 1, X + Y + 2  # v3b


def _angle_taps(k, var):
    """Flat arrays over valid taps: bin, samp, row, col, w (f32)."""
    y0, x0, wx, wy = _angle_tables(k)
    kk = np.broadcast_to(np.arange(S, dtype=np.int64)[:, None], (S, S))
    bb = np.broadcast_to(np.arange(S, dtype=np.int64)[None, :], (S, S))
    outs = []
    for dy in (0, 1):
        for dx in (0, 1):
            Y = y0 + dy
            X = x0 + dx
            w = (wy if dy else (1.0 - wy)) * (wx if dx else (1.0 - wx))
            v = (Y >= 0) & (Y < S) & (X >= 0) & (X < S)
            r, c = _variant_rc(var, Y, X)
            outs.append((bb[v], kk[v], r[v], c[v], w[v].astype(np.float32)))
    bins = np.concatenate([o[0] for o in outs])
    samp = np.concatenate([o[1] for o in outs])
    row = np.concatenate([o[2] for o in outs])
    col = np.concatenate([o[3] for o in outs])
    w = np.concatenate([o[4] for o in outs])
    return bins, samp, row, col, w


def _angle_tiles(k, var, d):
    """Per-bin ordered unique tiles + per-tap (group, slot, channel, elem).

    Returns dict with:
      lsegs[6]      per-chunk max segment length (ceil(ntiles/8))
      (after _plan pass B) idx/w scatter info
    """
    bins, samp, row, col, w = _angle_taps(k, var)
    rb = row // d
    xt = col // 8
    jc = col % 8
    e = row % d
    nrb = ROWS // d
    tile = xt * nrb + rb                       # block index in slab (< 4416)
    key = bins * (XT * nrb) + tile
    order = np.lexsort((samp, key))
    key_s = key[order]
    samp_s = samp[order]
    uk, first = np.unique(key_s, return_index=True)
    firstk = samp_s[first]                     # first sample touching tile
    ubin = uk // (XT * nrb)
    utile = uk % (XT * nrb)
    # order tiles within bin by first-sample
    o2 = np.lexsort((firstk, ubin))
    ubin2 = ubin[o2]
    utile2 = utile[o2]
    starts = np.searchsorted(ubin2, np.arange(S + 1))
    pos = np.arange(len(ubin2)) - starts[ubin2]
    ntiles = np.diff(starts)                   # tiles per bin
    # invert o2: for each unique-key row, its (group, t)
    grp_u = np.empty(len(uk), np.int64)
    t_u = np.empty(len(uk), np.int64)
    grp_u[o2] = pos % NGROUP
    t_u[o2] = pos // NGROUP
    # per-chunk lsegs
    edges = np.cumsum([0] + CHUNK_NXG)
    lsegs = []
    for ci in range(len(CHUNK_NXG)):
        lo, hi = edges[ci], min(edges[ci + 1], S)
        nt = ntiles[lo:hi] if hi > lo else np.zeros(1, np.int64)
        lsegs.append(int(np.ceil(nt.max() / NGROUP)) if len(nt) else 1)
    return dict(lsegs=[max(l, 1) for l in lsegs], uk=uk, ubin2=ubin2,
                utile2=utile2, grp_u=grp_u, t_u=t_u, key=key, jc=jc, e=e,
                w=w, bins=bins)


def _gather_cyc(nidx, d):
    return nidx * GATHER_CYC[d]


_PLAN_CACHE = {}


def _get_plan():
    if "plan" in _PLAN_CACHE:
        return _PLAN_CACHE["plan"]
    # pass A: per (core, angle, d): lsegs + cost -> per-core angle order,
    # global d_table / lseg_table.  All d are multiples of 8, so tile sets
    # for every d derive from one unique pass at d=8.
    info = {}
    for ci, (var, angles) in enumerate(CORE_SPECS):
        for k in angles:
            bins, samp, row, col, w = _angle_taps(k, var)
            key8 = (bins * XT + col // 8) * 48 + row // 8
            u8 = np.unique(key8)
            ubin8 = u8 // (48 * XT)
            uxt8 = (u8 // 48) % XT
            urb8 = u8 % 48
            per_d = {}
            for d in D_CHOICES:
                m = d // 8
                kd = (ubin8 * XT + uxt8) * 48 + urb8 // m
                ud = np.unique(kd)
                nt = np.bincount(ud // (48 * XT), minlength=S)
                nxgs = D_CHUNKS[d]
                edges = np.cumsum([0] + nxgs)
                lsegs = []
                for cix in range(len(nxgs)):
                    lo, hi = edges[cix], min(edges[cix + 1], S)
                    mx = nt[lo:hi].max() if hi > lo else 0
                    lsegs.append(max(int(np.ceil(mx / NGROUP)), 1))
                nidx = sum(nxg * l for nxg, l in zip(nxgs, lsegs))
                gat_ns = _gather_cyc(nidx, d) / 1.2
                dve_ns = nidx * d * DVE_NS_PER_EL
                per_d[d] = (lsegs, max(gat_ns, dve_ns), gat_ns, dve_ns)
            info[k] = per_d
    # per-core angle order: hardest first (by best-d cost)
    core_order = []
    for ci, (var, angles) in enumerate(CORE_SPECS):
        best = {k: min(v[d][1] for d in D_CHOICES) for k, v in
                ((k, info[k]) for k in angles)}
        core_order.append(sorted(angles, key=lambda k: -best[k]))

    # per-slot, per-d: cross-core max lsegs -> (gather_ns, dve_ns)
    def slot_cost(si, d):
        ls = [1] * len(D_CHUNKS[d])
        for ci in range(8):
            if si < len(core_order[ci]):
                al = info[core_order[ci][si]][d][0]
                ls = [max(a, b) for a, b in zip(ls, al)]
        nidx = sum(nxg * l for nxg, l in zip(D_CHUNKS[d], ls))
        gat = _gather_cyc(nidx, d) / 1.2 + CHUNK_OVERHEAD_NS * len(D_CHUNKS[d])
        return ls, gat, nidx * d * DVE_NS_PER_EL

    NSLOT_A = SLOTS - 1   # angle slots (slot 0 is the dummy priming slot)
    cost = {}
    for si in range(NSLOT_A):
        for d in D_CHOICES:
            ls, gat, dve = slot_cost(si, d)
            # cap per-chunk gathered volume so 4-deep DMA buffers fit SBUF
            if max(nxg * l * d for nxg, l in zip(D_CHUNKS[d], ls)) > MAXVOL:
                gat = dve = 1e18
            cost[(si, d)] = (ls, gat, dve)
    # choose d per slot minimizing max(total gather, total dve)
    d_table = [min(D_CHOICES, key=lambda d: cost[(si, d)][1])
               for si in range(NSLOT_A)]
    while True:
        G = sum(cost[(si, d_table[si])][1] for si in range(NSLOT_A))
        V = sum(cost[(si, d_table[si])][2] for si in range(NSLOT_A))
        best_move = None
        for si in range(NSLOT_A):
            for d in D_CHOICES:
                if d == d_table[si]:
                    continue
                g2 = G - cost[(si, d_table[si])][1] + cost[(si, d)][1]
                v2 = V - cost[(si, d_table[si])][2] + cost[(si, d)][2]
                m = max(g2, v2)
                if best_move is None or m < best_move[0]:
                    best_move = (m, si, d)
        if best_move is None or best_move[0] >= max(G, V) - 1.0:
            break
        d_table[best_move[1]] = best_move[2]
    lseg_table = [cost[(si, d_table[si])][0] for si in range(NSLOT_A)]
    d_table = [16] + d_table
    lseg_table = [[DUMMY_LSEG] * len(CHUNK_NXG)] + lseg_table
    # stream layout: per slot, per chunk: cn = nxg * lseg
    chunks = []
    o16 = ow = 0
    for si in range(SLOTS):
        d = d_table[si]
        nxgs = D_CHUNKS[d]
        for cidx, nxg in enumerate(nxgs):
            L = lseg_table[si][cidx]
            cn = nxg * L
            chunks.append(dict(si=si, cidx=cidx, d=d, L=L, cn=cn,
                               xoff=sum(nxgs[:cidx]), nxg=nxg,
                               o16=o16, ow=ow))
            o16 += cn // 16
            ow += cn * d
    tot16, totw = o16, ow
    maxcn = max(ch["cn"] for ch in chunks)
    maxels = max(ch["cn"] * ch["d"] for ch in chunks)

    # pass B: build per-core idx blobs [128, tot16] + w blobs [64, totw]
    core_idx = []
    core_w = []
    for ci, (var, angles) in enumerate(CORE_SPECS):
        idx_blob = np.zeros((128, tot16), np.int16)
        w_blob = np.zeros((64, totw), np.float32)
        for si in range(SLOTS):
            d = d_table[si]
            if si == 0 or si - 1 >= len(core_order[ci]):
                continue
            k = core_order[ci][si - 1]
            a = _angle_tiles(k, var, d)
            lsegs = lseg_table[si]
            sch = [c for c in chunks if c["si"] == si]
            nxgs = D_CHUNKS[d]
            edges = np.cumsum([0] + nxgs)
            # per-(bin,group) stream position of tile t:
            #   spos = chunk.o16*16 + (bin-lo)*L + t   (per-group stream)
            ub, ut = a["ubin2"], a["utile2"]
            # grp/t in o2 order: pos within bin
            starts = np.searchsorted(ub, np.arange(S + 1))
            pos = np.arange(len(ub)) - starts[ub]
            grp = pos % NGROUP
            tt = pos // NGROUP
            cid = np.searchsorted(edges, ub, side="right") - 1
            L_arr = np.array([lsegs[c] for c in range(len(nxgs))])
            off_arr = np.array([sch[c]["o16"] * 16 for c in range(len(nxgs))])
            assert np.all(tt < L_arr[cid]), (ci, si, k)
            spos = off_arr[cid] + (ub - edges[cid]) * L_arr[cid] + tt
            # scatter idx values: stream for group g wrapped into
            # partitions 16g..16g+15: idx[16g + (p%16), p//16] = val
            # default padding: repeat previous valid idx (avoid addr jumps)
            sv = np.zeros((NGROUP, tot16 * 16), np.int16)
            filled = np.zeros((NGROUP, tot16 * 16), bool)
            sv[grp, spos] = ut
            filled[grp, spos] = True
            # forward-fill padding within this slot's range
            lo16, hi16 = sch[0]["o16"] * 16, (sch[-1]["o16"] + sch[-1]["cn"] // 16) * 16
            for g in range(NGROUP):
                seg = sv[g, lo16:hi16]
                fil = filled[g, lo16:hi16]
                idxs = np.where(fil, np.arange(len(seg)), 0)
                np.maximum.accumulate(idxs, out=idxs)
                sv[g, lo16:hi16] = seg[idxs]
            # wrap into idx_blob
            for g in range(NGROUP):
                st = sv[g, lo16:hi16]
                wrap = st.reshape(-1, 16).T
                idx_blob[16 * g:16 * g + 16, lo16 // 16:hi16 // 16] = wrap
            # weights: per tap: blob row = 8*grp_tap + jc, col = spos*d + e
            tap_key = a["key"]
            urow = np.searchsorted(a["uk"], tap_key)
            tap_grp = a["grp_u"][urow]
            tap_t = a["t_u"][urow]
            tap_bin = a["bins"]
            tap_cid = np.searchsorted(edges, tap_bin, side="right") - 1
            tap_spos = (off_arr[tap_cid] + (tap_bin - edges[tap_cid])
                        * L_arr[tap_cid] + tap_t)
            wrow = 8 * tap_grp + a["jc"]
            ow_arr = np.array([sch[c]["ow"] for c in range(len(nxgs))])
            o16_arr = np.array([sch[c]["o16"] * 16 for c in range(len(nxgs))])
            wcol = ow_arr[tap_cid] + (tap_spos - o16_arr[tap_cid]) * d + a["e"]
            np.add.at(w_blob, (wrow, wcol), a["w"])
        core_idx.append(idx_blob)
        core_w.append(w_blob.astype(BF16))

    sel = np.zeros((128, 2), np.float32)
    for p in range(128):
        sel[p, p % 2] = 1.0
    plan = dict(d_table=d_table, lseg_table=lseg_table, chunks=chunks,
                tot16=tot16, totw=totw, maxcn=maxcn, maxels=maxels,
                core_idx=core_idx, core_w=core_w, sel=sel,
                core_order=core_order)
    _PLAN_CACHE["plan"] = plan
    return plan


def _build_slab(image, var):
    """[128, NELS] bf16: partition p=(g,jc,b): cols ≡ jc mod 8 of variant
    frame, batch b; element idx = xt*ROWS + row."""
    img = np.asarray(image, np.float32)[:, 0]
    # padded image: 256 -> 363
    pad_total = S - IMG_SIZE
    pb = pad_total // 2
    pimg = np.zeros((BATCH, S, S), np.float32)
    pimg[:, pb:pb + IMG_SIZE, pb:pb + IMG_SIZE] = img
    Yg, Xg = np.meshgrid(np.arange(S), np.arange(S), indexing="ij")
    r, c = _variant_rc(var, Yg, Xg)
    frame = np.zeros((BATCH, ROWS, NCOL), np.float32)
    frame[:, r, c] = pimg
    # slice: [16, XT, ROWS]
    slab16 = np.zeros((16, NELS), np.float32)
    for jc in range(8):
        cols = frame[:, :, jc::8]              # [B, ROWS, XT]
        sl = np.transpose(cols, (0, 2, 1)).reshape(BATCH, -1)  # xt-major rows
        for b in range(BATCH):
            slab16[2 * jc + b] = sl[b]
    return np.tile(slab16, (8, 1)).astype(BF16)


_PROG_CACHE = {}


def _build_program(plan):
    if "prog" in _PROG_CACHE:
        return _PROG_CACHE["prog"]
    import concourse.bass as bass
    import concourse.mybir as mybir
    from concourse import library_config

    chunks = plan["chunks"]
    maxcn = plan["maxcn"]
    maxels = plan["maxels"]

    nc = bass.Bass()
    slab_d = nc.declare_dram_parameter("slab", [128, NELS],
                                       mybir.dt.bfloat16, isOutput=False)
    idx_d = nc.declare_dram_parameter("idx", [128, plan["tot16"]],
                                      mybir.dt.int16, isOutput=False)
    w_d = nc.declare_dram_parameter("w", [64, plan["totw"]],
                                    mybir.dt.bfloat16, isOutput=False)
    sel_d = nc.declare_dram_parameter("sel", [128, 2], mybir.dt.float32,
                                      isOutput=False)
    out_d = nc.declare_dram_parameter("out", [SLOTS, 2, SXPAD],
                                      mybir.dt.float32, isOutput=True)

    ctx = ExitStack()
    with ctx:
        slab_t = ctx.enter_context(nc.sbuf_tensor([128, NELS], mybir.dt.bfloat16))
        idx_ts = [ctx.enter_context(nc.sbuf_tensor(f"idx{i}", [128, maxcn // 16], mybir.dt.int16)) for i in range(4)]
        w_ts = [ctx.enter_context(nc.sbuf_tensor(f"w{i}", [128, maxels], mybir.dt.bfloat16)) for i in range(4)]
        g_ts = [ctx.enter_context(nc.sbuf_tensor(f"g{i}", [128, maxels], mybir.dt.bfloat16)) for i in range(3)]
        p_t = ctx.enter_context(nc.sbuf_tensor([128, maxels], mybir.dt.bfloat16))
        r_ts = [ctx.enter_context(nc.sbuf_tensor(f"r{i}", [128, SXPAD], mybir.dt.float32)) for i in range(2)]
        sel_t = ctx.enter_context(nc.sbuf_tensor([128, 2], mybir.dt.float32))
        vscr_t = ctx.enter_context(nc.sbuf_tensor([128, 2], mybir.dt.float32))
        sino_ts = [ctx.enter_context(nc.sbuf_tensor(f"sino{i}", [2, SXPAD], mybir.dt.float32)) for i in range(2)]
        psum_ts = [ctx.enter_context(nc.psum_tensor(f"ps{i}", [2, SXPAD], mybir.dt.float32)) for i in range(2)]
        s_in = ctx.enter_context(nc.semaphore("s_in"))
        s_dma = ctx.enter_context(nc.semaphore("s_dma"))
        s_g = ctx.enter_context(nc.semaphore("s_g"))
        s_v = ctx.enter_context(nc.semaphore("s_v"))
        s_mm = ctx.enter_context(nc.semaphore("s_mm"))
        s_cp = ctx.enter_context(nc.semaphore("s_cp"))
        s_od = ctx.enter_context(nc.semaphore("s_od"))
        block = ctx.enter_context(nc.Block())

        slot_end = [0] * SLOTS
        for n, ch in enumerate(chunks):
            slot_end[ch["si"]] = n + 1

        @block.sync
        def _(sync):
            sync.dma_start(out=sel_t[:], in_=sel_d[:]).then_inc(s_in, 16)
            sync.dma_start(out=slab_t[:], in_=slab_d[:]).then_inc(s_in, 16)
            for n, ch in enumerate(chunks):
                # 4-deep prefetch: buffer n%4 was last used by chunk n-4
                # (idx read by gather n-4, w read by vector n-4); the deep
                # pipeline also gives cold-start DMAs time to actually land
                # (completion semaphores fire early).
                if n > 3:
                    sync.wait_ge(s_g, n - 3)
                    sync.wait_ge(s_v, n - 3)
                sync.dma_start(
                    out=idx_ts[n % 4][:, :ch["cn"] // 16],
                    in_=idx_d[:, ch["o16"]:ch["o16"] + ch["cn"] // 16],
                ).then_inc(s_dma, 16)
                cnd = ch["cn"] * ch["d"]
                wsrc = (w_d[:, ch["ow"]:ch["ow"] + cnd]
                        .unsqueeze(1).broadcast_to([64, 2, cnd]))
                sync.dma_start(out=w_ts[n % 4][:, :cnd], in_=wsrc).then_inc(s_dma, 16)

        @block.gpsimd
        def _(g):
            g.load_library(library_config.ap_gather)
            g.wait_ge(s_in, 32)
            g.wait_ge(s_dma, 32)
            # warmup: amortize ext-isa first-call cost + preamble margin
            ch0 = chunks[0]
            d0 = ch0["d"]
            for _ in range(1):
                g.ap_gather(
                    g_ts[2][:, :ch0["cn"] * d0].rearrange("p (n d) -> p n d", d=d0),
                    slab_t[:].rearrange("p (n d) -> p n d", d=d0),
                    idx_ts[0][:, :ch0["cn"] // 16],
                    channels=128, num_elems=NELS // d0, d=d0, num_idxs=ch0["cn"],
                )
            for n, ch in enumerate(chunks):
                d = ch["d"]
                g.wait_ge(s_dma, 32 * (n + 1))
                if n > 2:
                    g.wait_ge(s_v, n - 2)  # g_ts[n%3] consumed by vector n-3
                g.ap_gather(
                    g_ts[n % 3][:, :ch["cn"] * d].rearrange("p (n d) -> p n d", d=d),
                    slab_t[:].rearrange("p (n d) -> p n d", d=d),
                    idx_ts[n % 4][:, :ch["cn"] // 16],
                    channels=128, num_elems=NELS // d, d=d, num_idxs=ch["cn"],
                ).then_inc(s_g, 1)

        @block.vector
        def _(v):
            for n, ch in enumerate(chunks):
                v.wait_ge(s_g, n + 1)
                if ch["cidx"] == 0 and ch["si"] > 1:
                    v.wait_ge(s_mm, ch["si"] - 1)
                cnd = ch["cn"] * ch["d"]
                v.tensor_mul(p_t[:, :cnd], g_ts[n % 3][:, :cnd],
                             w_ts[n % 4][:, :cnd])
                rdst = r_ts[ch["si"] % 2]
                v.tensor_reduce(
                    out=rdst[:, ch["xoff"]:ch["xoff"] + ch["nxg"]],
                    in_=p_t[:, :cnd].rearrange(
                        "p (x l) -> p x l", l=ch["L"] * ch["d"]),
                    axis=mybir.AxisListType.X,
                    op=mybir.AluOpType.add,
                )
                v.tensor_copy(vscr_t[:, :1],
                              rdst[:, ch["xoff"]:ch["xoff"] + 1]).then_inc(s_v, 1)

        @block.tensor
        def _(t):
            t.wait_ge(s_in, 32)
            for si in range(SLOTS):
                t.wait_ge(s_v, slot_end[si])
                if si > 1:
                    t.wait_ge(s_cp, si - 1)
                t.matmul(psum_ts[si % 2][:], sel_t[:], r_ts[si % 2][:],
                         start=True, stop=True).then_inc(s_mm, 1)

        @block.scalar
        def _(sc):
            for si in range(SLOTS):
                sc.wait_ge(s_mm, si + 1)
                if si > 1:
                    sc.wait_ge(s_od, 16 * (si - 1))  # sino buf freed by DMA
                sc.copy(sino_ts[si % 2][:], psum_ts[si % 2][:]).then_inc(s_cp, 1)
                sc.dma_start(out=out_d[si], in_=sino_ts[si % 2][:]
                             ).then_inc(s_od, 16)
            sc.wait_ge(s_od, 16 * SLOTS)

    import concourse.mybir as mybir2
    mybir2.codegen_inst_isa_subclasses(nc)
    _PROG_CACHE["prog"] = nc
    return nc


def kernel(image):
    image = np.asarray(image, np.float32)
    assert image.shape == (BATCH, 1, IMG_SIZE, IMG_SIZE)
    plan = _get_plan()
    nc = _build_program(plan)

    from concourse.bass_utils import run_bass_kernel_spmd

    in_maps = []
    for ci, (var, angles) in enumerate(CORE_SPECS):
        in_maps.append({
            "slab": _build_slab(image, var),
            "idx": plan["core_idx"][ci],
            "w": plan["core_w"][ci],
            "sel": plan["sel"],
        })

    trace = bool(os.environ.get("RADON_TRACE"))
    if trace:
        _install_profhook()
    res = run_bass_kernel_spmd(nc, in_maps, list(range(8)), trace=trace)
    if trace:
        kernel.last_exec_time_ns = res.exec_time_ns

    sino = np.zeros((BATCH, 1, S, N_ANGLES), np.float32)
    for ci in range(8):
        o = res.results[ci]["out"]  # [SLOTS, 2, SXPAD]; slot 0 is dummy
        for si, k in enumerate(plan["core_order"][ci]):
            sino[:, 0, :, k] = o[si + 1, :, :S]
    return sino


def _install_profhook():
    import types
    if "antenv.axon_hooks" in sys.modules:
        return
    try:
        from trn_agent_boot.trn_boot import _ntff_profile_via_ctypes
        hook = _ntff_profile_via_ctypes("/opt/axon/libaxon_pjrt.so")
    except Exception:
        hook = None
    mod = types.ModuleType("antenv.axon_hooks")
    mod._hook = hook
    mod.set_axon_ntff_profile_hook = lambda h: setattr(mod, "_hook", h)
    mod.get_axon_ntff_profile_hook = lambda: mod._hook
    sys.modules["antenv.axon_hooks"] = mod
    import antenv
    antenv.axon_hooks = mod


if __name__ == "__main__":
    img = np.load("/tmp/ref_image.npy")
    out = kernel(image=img)
    exp = np.load("/tmp/ref_expected.npy")
    err = np.linalg.norm(out - exp) / np.linalg.norm(exp)
    print("kernel rel err:", err)


# revision 46
# speedup vs baseline: 1.0425x; 1.0102x over previous
"""Radon transform (bilinear grid-sample + row-sum) on 8 TRN2 NeuronCores.

Tile-gather design: each angle's sample lines are walked in a frame variant
(identity / transpose / +-diagonal shear) chosen so the line advances < 0.59
columns per row.  The frame is stored column-sliced across each 16-partition
GPSIMD group (partition = (col mod 8, batch)); one ap_gather index fetches a
d x 8 pixel tile (d rows x 8 adjacent columns x 2 batches) in one shot, so
the per-index RD_CMD latency (the kernel bottleneck) is amortized over a
whole 2-D tile instead of a single bilinear tap pair.  All 8 groups hold
identical slabs; a bin's tiles are dealt round-robin to groups.  DVE applies
precomputed weights + segment-reduce per sinogram bin; a ones-ish matmul
sums partitions; per-slot DMA returns [2, 368] sinogram columns.

All gather indices / weights are input-independent and precomputed on host.
"""
import math
import os
import sys
from contextlib import ExitStack

import numpy as np

sys.path.insert(0, "/opt/trn_rl_repo")

import ml_dtypes  # noqa: E402

BF16 = ml_dtypes.bfloat16

# ─── geometry constants (hardcoded for 256x256, 180 angles, batch 2) ───
N_ANGLES = 180
IMG_SIZE = 256
BATCH = 2
S = int(math.ceil(math.sqrt(2.0) * IMG_SIZE))  # 363

ROWS = 384            # slab rows (multiple of 32)
XT = 92               # col-tiles per partition slice
NELS = XT * ROWS      # 35328 elements per partition
NCOL = XT * 8         # 736 layout columns
SXPAD = 368           # padded bin count
SLOTS = 24            # slot 0 is a dummy pipeline-priming slot
DUMMY_LSEG = 2
NGROUP = 8
D_CHOICES = (8, 16, 24, 32, 48)
# per-d bin chunking: nxg multiples of 16 (index wrap), sized so
# nxg*lseg*d stays under the SBUF chunk-volume cap even at large lseg
D_CHUNKS = {
    8: [64, 64, 64, 64, 64, 48],
    16: [64, 64, 64, 64, 64, 48],
    24: [64, 64, 64, 64, 64, 48],
    32: [64, 64, 64, 64, 64, 48],
    48: [64, 64, 64, 64, 64, 48],
}
CHUNK_NXG = D_CHUNKS[16]  # dummy-slot scheme
MAXVOL = 8192
# hardware-measured per-index gather cost (cycles) by block depth d
GATHER_CYC = {2: 33.0, 8: 40.0, 16: 39.5, 24: 56.8, 32: 62.3, 48: 76.0}
DVE_NS_PER_EL = 1.33     # mult (2.2 el/ns) + segment reduce (1.13 el/ns)
CHUNK_OVERHEAD_NS = 0     # per-chunk gather dispatch/idx-load/sync cost

# angle classes: (variant, angle list). variant row/col maps:
#   v0: row=Y+1, col=X+1            (|tan| <= tan22.5, theta near 0/180)
#   v1: row=X+1, col=Y+1            (theta near 90)
#   v2a: row=Y+1, col=X-Y+368       (22.5 < th <= 45)
#   v2b: row=X+1, col=X-Y+368       (45 < th < 67.5)
#   v3a: row=X+1, col=X+Y+2         (112.5 < th <= 135)
#   v3b: row=Y+1, col=X+Y+2         (135 < th < 157.5)
CORE_SPECS = [
    ("v0", list(range(0, 23))),
    ("v0", list(range(158, 180))),
    ("v2a", list(range(23, 46))),
    ("v2b", list(range(46, 68))),
    ("v1", list(range(68, 91))),
    ("v1", list(range(91, 113))),
    ("v3a", list(range(113, 136))),
    ("v3b", list(range(136, 158))),
]


def _angle_tables(k):
    theta = np.float32(k) * np.float32(np.pi / N_ANGLES)
    c = np.cos(theta, dtype=np.float32)
    s = np.sin(theta, dtype=np.float32)
    lin = np.linspace(-1.0, 1.0, S, dtype=np.float32)
    x = lin[None, :]
    y = lin[:, None]
    gx = c * x + s * y
    gy = -s * x + c * y
    ix = ((gx + np.float32(1.0)) * np.float32(0.5) * np.float32(S - 1)).astype(np.float32)
    iy = ((gy + np.float32(1.0)) * np.float32(0.5) * np.float32(S - 1)).astype(np.float32)
    x0 = np.floor(ix)
    y0 = np.floor(iy)
    wx = ix - x0
    wy = iy - y0
    return y0.astype(np.int64), x0.astype(np.int64), wx, wy


def _variant_rc(var, Y, X):
    if var == "v0":
        return Y + 1, X + 1
    if var == "v1":
        return X + 1, Y + 1
    if var == "v2a":
        return Y + 1, X - Y + 368
    if var == "v2b":
        return X + 1, X - Y + 368
    if var == "v3a":
        return X + 1, X + Y + 2
    return Y + 1, X + Y + 2  # v3b


def _angle_taps(k, var):
    """Flat arrays over valid taps: bin, samp, row, col, w (f32)."""
    y0, x0, wx, wy = _angle_tables(k)
    kk = np.broadcast_to(np.arange(S, dtype=np.int64)[:, None], (S, S))
    bb = np.broadcast_to(np.arange(S, dtype=np.int64)[None, :], (S, S))
    outs = []
    for dy in (0, 1):
        for dx in (0, 1):
            Y = y0 + dy
            X = x0 + dx
            w = (wy if dy else (1.0 - wy)) * (wx if dx else (1.0 - wx))
            v = (Y >= 0) & (Y < S) & (X >= 0) & (X < S)
            r, c = _variant_rc(var, Y, X)
            outs.append((bb[v], kk[v], r[v], c[v], w[v].astype(np.float32)))
    bins = np.concatenate([o[0] for o in outs])
    samp = np.concatenate([o[1] for o in outs])
    row = np.concatenate([o[2] for o in outs])
    col = np.concatenate([o[3] for o in outs])
    w = np.concatenate([o[4] for o in outs])
    return bins, samp, row, col, w


def _angle_tiles(k, var, d):
    """Per-bin ordered unique tiles + per-tap (group, slot, channel, elem).

    Returns dict with:
      lsegs[6]      per-chunk max segment length (ceil(ntiles/8))
      (after _plan pass B) idx/w scatter info
    """
    bins, samp, row, col, w = _angle_taps(k, var)
    rb = row // d
    xt = col // 8
    jc = col % 8
    e = row % d
    nrb = ROWS // d
    tile = xt * nrb + rb                       # block index in slab (< 4416)
    key = bins * (XT * nrb) + tile
    order = np.lexsort((samp, key))
    key_s = key[order]
    samp_s = samp[order]
    uk, first = np.unique(key_s, return_index=True)
    firstk = samp_s[first]                     # first sample touching tile
    ubin = uk // (XT * nrb)
    utile = uk % (XT * nrb)
    # order tiles within bin by first-sample
    o2 = np.lexsort((firstk, ubin))
    ubin2 = ubin[o2]
    utile2 = utile[o2]
    starts = np.searchsorted(ubin2, np.arange(S + 1))
    pos = np.arange(len(ubin2)) - starts[ubin2]
    ntiles = np.diff(starts)                   # tiles per bin
    # invert o2: for each unique-key row, its (group, t)
    grp_u = np.empty(len(uk), np.int64)
    t_u = np.empty(len(uk), np.int64)
    grp_u[o2] = pos % NGROUP
    t_u[o2] = pos // NGROUP
    # per-chunk lsegs
    edges = np.cumsum([0] + CHUNK_NXG)
    lsegs = []
    for ci in range(len(CHUNK_NXG)):
        lo, hi = edges[ci], min(edges[ci + 1], S)
        nt = ntiles[lo:hi] if hi > lo else np.zeros(1, np.int64)
        lsegs.append(int(np.ceil(nt.max() / NGROUP)) if len(nt) else 1)
    return dict(lsegs=[max(l, 1) for l in lsegs], uk=uk, ubin2=ubin2,
                utile2=utile2, grp_u=grp_u, t_u=t_u, key=key, jc=jc, e=e,
                w=w, bins=bins)


def _gather_cyc(nidx, d):
    return nidx * GATHER_CYC[d]


_PLAN_CACHE = {}


def _get_plan():
    if "plan" in _PLAN_CACHE:
        return _PLAN_CACHE["plan"]
    # pass A: per (core, angle, d): lsegs + cost -> per-core angle order,
    # global d_table / lseg_table.  All d are multiples of 8, so tile sets
    # for every d derive from one unique pass at d=8.
    info = {}
    for ci, (var, angles) in enumerate(CORE_SPECS):
        for k in angles:
            bins, samp, row, col, w = _angle_taps(k, var)
            key8 = (bins * XT + col // 8) * 48 + row // 8
            u8 = np.unique(key8)
            ubin8 = u8 // (48 * XT)
            uxt8 = (u8 // 48) % XT
            urb8 = u8 % 48
            per_d = {}
            for d in D_CHOICES:
                m = d // 8
                kd = (ubin8 * XT + uxt8) * 48 + urb8 // m
                ud = np.unique(kd)
                nt = np.bincount(ud // (48 * XT), minlength=S)
                nxgs = D_CHUNKS[d]
                edges = np.cumsum([0] + nxgs)
                lsegs = []
                for cix in range(len(nxgs)):
                    lo, hi = edges[cix], min(edges[cix + 1], S)
                    mx = nt[lo:hi].max() if hi > lo else 0
                    lsegs.append(max(int(np.ceil(mx / NGROUP)), 1))
                nidx = sum(nxg * l for nxg, l in zip(nxgs, lsegs))
                gat_ns = _gather_cyc(nidx, d) / 1.2
                dve_ns = nidx * d * DVE_NS_PER_EL
                per_d[d] = (lsegs, max(gat_ns, dve_ns), gat_ns, dve_ns)
            info[k] = per_d
    # per-core angle order: hardest first (by best-d cost)
    core_order = []
    for ci, (var, angles) in enumerate(CORE_SPECS):
        best = {k: min(v[d][1] for d in D_CHOICES) for k, v in
                ((k, info[k]) for k in angles)}
        core_order.append(sorted(angles, key=lambda k: -best[k]))

    # per-slot, per-d: cross-core max lsegs -> (gather_ns, dve_ns)
    def slot_cost(si, d):
        ls = [1] * len(D_CHUNKS[d])
        for ci in range(8):
            if si < len(core_order[ci]):
                al = info[core_order[ci][si]][d][0]
                ls = [max(a, b) for a, b in zip(ls, al)]
        nidx = sum(nxg * l for nxg, l in zip(D_CHUNKS[d], ls))
        gat = _gather_cyc(nidx, d) / 1.2 + CHUNK_OVERHEAD_NS * len(D_CHUNKS[d])
        return ls, gat, nidx * d * DVE_NS_PER_EL

    NSLOT_A = SLOTS - 1   # angle slots (slot 0 is the dummy priming slot)
    cost = {}
    for si in range(NSLOT_A):
        for d in D_CHOICES:
            ls, gat, dve = slot_cost(si, d)
            # cap per-chunk gathered volume so 4-deep DMA buffers fit SBUF
            if max(nxg * l * d for nxg, l in zip(D_CHUNKS[d], ls)) > MAXVOL:
                gat = dve = 1e18
            cost[(si, d)] = (ls, gat, dve)
    # choose d per slot minimizing max(total gather, total dve)
    d_table = [min(D_CHOICES, key=lambda d: cost[(si, d)][1])
               for si in range(NSLOT_A)]
    while True:
        G = sum(cost[(si, d_table[si])][1] for si in range(NSLOT_A))
        V = sum(cost[(si, d_table[si])][2] for si in range(NSLOT_A))
        best_move = None
        for si in range(NSLOT_A):
            for d in D_CHOICES:
                if d == d_table[si]:
                    continue
                g2 = G - cost[(si, d_table[si])][1] + cost[(si, d)][1]
                v2 = V - cost[(si, d_table[si])][2] + cost[(si, d)][2]
                m = max(g2, v2)
                if best_move is None or m < best_move[0]:
                    best_move = (m, si, d)
        if best_move is None or best_move[0] >= max(G, V) - 1.0:
            break
        d_table[best_move[1]] = best_move[2]
    lseg_table = [cost[(si, d_table[si])][0] for si in range(NSLOT_A)]
    d_table = [16] + d_table
    lseg_table = [[DUMMY_LSEG] * len(CHUNK_NXG)] + lseg_table
    # stream layout: per slot, per chunk: cn = nxg * lseg
    chunks = []
    o16 = ow = 0
    for si in range(SLOTS):
        d = d_table[si]
        nxgs = D_CHUNKS[d]
        for cidx, nxg in enumerate(nxgs):
            L = lseg_table[si][cidx]
            cn = nxg * L
            chunks.append(dict(si=si, cidx=cidx, d=d, L=L, cn=cn,
                               xoff=sum(nxgs[:cidx]), nxg=nxg,
                               o16=o16, ow=ow))
            o16 += cn // 16
            ow += cn * d
    tot16, totw = o16, ow
    maxcn = max(ch["cn"] for ch in chunks)
    maxels = max(ch["cn"] * ch["d"] for ch in chunks)

    # pass B: build per-core idx blobs [128, tot16] + w blobs [64, totw]
    core_idx = []
    core_w = []
    for ci, (var, angles) in enumerate(CORE_SPECS):
        idx_blob = np.zeros((128, tot16), np.int16)
        w_blob = np.zeros((64, totw), np.float32)
        for si in range(SLOTS):
            d = d_table[si]
            if si == 0 or si - 1 >= len(core_order[ci]):
                continue
            k = core_order[ci][si - 1]
            a = _angle_tiles(k, var, d)
            lsegs = lseg_table[si]
            sch = [c for c in chunks if c["si"] == si]
            nxgs = D_CHUNKS[d]
            edges = np.cumsum([0] + nxgs)
            # per-(bin,group) stream position of tile t:
            #   spos = chunk.o16*16 + (bin-lo)*L + t   (per-group stream)
            ub, ut = a["ubin2"], a["utile2"]
            # grp/t in o2 order: pos within bin
            starts = np.searchsorted(ub, np.arange(S + 1))
            pos = np.arange(len(ub)) - starts[ub]
            grp = pos % NGROUP
            tt = pos // NGROUP
            cid = np.searchsorted(edges, ub, side="right") - 1
            L_arr = np.array([lsegs[c] for c in range(len(nxgs))])
            off_arr = np.array([sch[c]["o16"] * 16 for c in range(len(nxgs))])
            assert np.all(tt < L_arr[cid]), (ci, si, k)
            spos = off_arr[cid] + (ub - edges[cid]) * L_arr[cid] + tt
            # scatter idx values: stream for group g wrapped into
            # partitions 16g..16g+15: idx[16g + (p%16), p//16] = val
            # default padding: repeat previous valid idx (avoid addr jumps)
            sv = np.zeros((NGROUP, tot16 * 16), np.int16)
            filled = np.zeros((NGROUP, tot16 * 16), bool)
            sv[grp, spos] = ut
            filled[grp, spos] = True
            # forward-fill padding within this slot's range
            lo16, hi16 = sch[0]["o16"] * 16, (sch[-1]["o16"] + sch[-1]["cn"] // 16) * 16
            for g in range(NGROUP):
                seg = sv[g, lo16:hi16]
                fil = filled[g, lo16:hi16]
                idxs = np.where(fil, np.arange(len(seg)), 0)
                np.maximum.accumulate(idxs, out=idxs)
                sv[g, lo16:hi16] = seg[idxs]
            # wrap into idx_blob
            for g in range(NGROUP):
                st = sv[g, lo16:hi16]
                wrap = st.reshape(-1, 16).T
                idx_blob[16 * g:16 * g + 16, lo16 // 16:hi16 // 16] = wrap
            # weights: per tap: blob row = 8*grp_tap + jc, col = spos*d + e
            tap_key = a["key"]
            urow = np.searchsorted(a["uk"], tap_key)
            tap_grp = a["grp_u"][urow]
            tap_t = a["t_u"][urow]
            tap_bin = a["bins"]
            tap_cid = np.searchsorted(edges, tap_bin, side="right") - 1
            tap_spos = (off_arr[tap_cid] + (tap_bin - edges[tap_cid])
                        * L_arr[tap_cid] + tap_t)
            wrow = 8 * tap_grp + a["jc"]
            ow_arr = np.array([sch[c]["ow"] for c in range(len(nxgs))])
            o16_arr = np.array([sch[c]["o16"] * 16 for c in range(len(nxgs))])
            wcol = ow_arr[tap_cid] + (tap_spos - o16_arr[tap_cid]) * d + a["e"]
            np.add.at(w_blob, (wrow, wcol), a["w"])
        core_idx.append(idx_blob)
        core_w.append(w_blob.astype(BF16))

    sel = np.zeros((128, 2), np.float32)
    for p in range(128):
        sel[p, p % 2] = 1.0
    plan = dict(d_table=d_table, lseg_table=lseg_table, chunks=chunks,
                tot16=tot16, totw=totw, maxcn=maxcn, maxels=maxels,
                core_idx=core_idx, core_w=core_w, sel=sel,
                core_order=core_order)
    _PLAN_CACHE["plan"] = plan
    return plan


def _build_slab(image, var):
    """[128, NELS] bf16: partition p=(g,jc,b): cols ≡ jc mod 8 of variant
    frame, batch b; element idx = xt*ROWS + row."""
    img = np.asarray(image, np.float32)[:, 0]
    # padded image: 256 -> 363
    pad_total = S - IMG_SIZE
    pb = pad_total // 2
    pimg = np.zeros((BATCH, S, S), np.float32)
    pimg[:, pb:pb + IMG_SIZE, pb:pb + IMG_SIZE] = img
    Yg, Xg = np.meshgrid(np.arange(S), np.arange(S), indexing="ij")
    r, c = _variant_rc(var, Yg, Xg)
    frame = np.zeros((BATCH, ROWS, NCOL), np.float32)
    frame[:, r, c] = pimg
    # slice: [16, XT, ROWS]
    slab16 = np.zeros((16, NELS), np.float32)
    for jc in range(8):
        cols = frame[:, :, jc::8]              # [B, ROWS, XT]
        sl = np.transpose(cols, (0, 2, 1)).reshape(BATCH, -1)  # xt-major rows
        for b in range(BATCH):
            slab16[2 * jc + b] = sl[b]
    return np.tile(slab16, (8, 1)).astype(BF16)


_PROG_CACHE = {}


def _build_program(plan):
    if "prog" in _PROG_CACHE:
        return _PROG_CACHE["prog"]
    import concourse.bass as bass
    import concourse.mybir as mybir
    from concourse import library_config

    chunks = plan["chunks"]
    maxcn = plan["maxcn"]
    maxels = plan["maxels"]

    nc = bass.Bass()
    slab_d = nc.declare_dram_parameter("slab", [128, NELS],
                                       mybir.dt.bfloat16, isOutput=False)
    idx_d = nc.declare_dram_parameter("idx", [128, plan["tot16"]],
                                      mybir.dt.int16, isOutput=False)
    w_d = nc.declare_dram_parameter("w", [64, plan["totw"]],
                                    mybir.dt.bfloat16, isOutput=False)
    sel_d = nc.declare_dram_parameter("sel", [128, 2], mybir.dt.float32,
                                      isOutput=False)
    out_d = nc.declare_dram_parameter("out", [SLOTS, 2, SXPAD],
                                      mybir.dt.float32, isOutput=True)

    ctx = ExitStack()
    with ctx:
        slab_t = ctx.enter_context(nc.sbuf_tensor([128, NELS], mybir.dt.bfloat16))
        idx_ts = [ctx.enter_context(nc.sbuf_tensor(f"idx{i}", [128, maxcn // 16], mybir.dt.int16)) for i in range(4)]
        w_ts = [ctx.enter_context(nc.sbuf_tensor(f"w{i}", [128, maxels], mybir.dt.bfloat16)) for i in range(4)]
        g_ts = [ctx.enter_context(nc.sbuf_tensor(f"g{i}", [128, maxels], mybir.dt.bfloat16)) for i in range(3)]
        p_t = ctx.enter_context(nc.sbuf_tensor([128, maxels], mybir.dt.bfloat16))
        r_ts = [ctx.enter_context(nc.sbuf_tensor(f"r{i}", [128, SXPAD], mybir.dt.float32)) for i in range(2)]
        sel_t = ctx.enter_context(nc.sbuf_tensor([128, 2], mybir.dt.float32))
        vscr_t = ctx.enter_context(nc.sbuf_tensor([128, 2], mybir.dt.float32))
        sino_ts = [ctx.enter_context(nc.sbuf_tensor(f"sino{i}", [2, SXPAD], mybir.dt.float32)) for i in range(2)]
        psum_ts = [ctx.enter_context(nc.psum_tensor(f"ps{i}", [2, SXPAD], mybir.dt.float32)) for i in range(2)]
        s_in = ctx.enter_context(nc.semaphore("s_in"))
        s_dma = ctx.enter_context(nc.semaphore("s_dma"))
        s_g = ctx.enter_context(nc.semaphore("s_g"))
        s_v = ctx.enter_context(nc.semaphore("s_v"))
        s_mm = ctx.enter_context(nc.semaphore("s_mm"))
        s_cp = ctx.enter_context(nc.semaphore("s_cp"))
        s_od = ctx.enter_context(nc.semaphore("s_od"))
        block = ctx.enter_context(nc.Block())

        slot_end = [0] * SLOTS
        for n, ch in enumerate(chunks):
            slot_end[ch["si"]] = n + 1

        @block.sync
        def _(sync):
            sync.dma_start(out=sel_t[:], in_=sel_d[:]).then_inc(s_in, 16)
            sync.dma_start(out=slab_t[:], in_=slab_d[:]).then_inc(s_in, 16)
            for n, ch in enumerate(chunks):
                # 4-deep prefetch: buffer n%4 was last used by chunk n-4
                # (idx read by gather n-4, w read by vector n-4); the deep
                # pipeline also gives cold-start DMAs time to actually land
                # (completion semaphores fire early).
                if n > 3:
                    sync.wait_ge(s_g, n - 3)
                    sync.wait_ge(s_v, n - 3)
                sync.dma_start(
                    out=idx_ts[n % 4][:, :ch["cn"] // 16],
                    in_=idx_d[:, ch["o16"]:ch["o16"] + ch["cn"] // 16],
                ).then_inc(s_dma, 16)
                cnd = ch["cn"] * ch["d"]
                wsrc = (w_d[:, ch["ow"]:ch["ow"] + cnd]
                        .unsqueeze(1).broadcast_to([64, 2, cnd]))
                sync.dma_start(out=w_ts[n % 4][:, :cnd], in_=wsrc).then_inc(s_dma, 16)

        @block.gpsimd
        def _(g):
            g.load_library(library_config.ap_gather)
            g.wait_ge(s_in, 32)
            g.wait_ge(s_dma, 32)
            # warmup: amortize ext-isa first-call cost + preamble margin
            ch0 = chunks[0]
            d0 = ch0["d"]
            for _ in range(1):
                g.ap_gather(
                    g_ts[2][:, :ch0["cn"] * d0].rearrange("p (n d) -> p n d", d=d0),
                    slab_t[:].rearrange("p (n d) -> p n d", d=d0),
                    idx_ts[0][:, :ch0["cn"] // 16],
                    channels=128, num_elems=NELS // d0, d=d0, num_idxs=ch0["cn"],
                )
            for n, ch in enumerate(chunks):
                d = ch["d"]
                g.wait_ge(s_dma, 32 * (n + 1))
                if n > 2:
                    g.wait_ge(s_v, n - 2)  # g_ts[n%3] consumed by vector n-3
                g.ap_gather(
                    g_ts[n % 3][:, :ch["cn"] * d].rearrange("p (n d) -> p n d", d=d),
                    slab_t[:].rearrange("p (n d) -> p n d", d=d),
                    idx_ts[n % 4][:, :ch["cn"] // 16],
                    channels=128, num_elems=NELS // d, d=d, num_idxs=ch["cn"],
                ).then_inc(s_g, 1)

        @block.vector
        def _(v):
            for n, ch in enumerate(chunks):
                v.wait_ge(s_g, n + 1)
                if ch["cidx"] == 0 and ch["si"] > 1:
                    v.wait_ge(s_mm, ch["si"] - 1)
                cnd = ch["cn"] * ch["d"]
                v.tensor_mul(p_t[:, :cnd], g_ts[n % 3][:, :cnd],
                             w_ts[n % 4][:, :cnd])
                rdst = r_ts[ch["si"] % 2]
                v.tensor_reduce(
                    out=rdst[:, ch["xoff"]:ch["xoff"] + ch["nxg"]],
                    in_=p_t[:, :cnd].rearrange(
                        "p (x l) -> p x l", l=ch["L"] * ch["d"]),
                    axis=mybir.AxisListType.X,
                    op=mybir.AluOpType.add,
                )
                v.tensor_copy(vscr_t[:, :1],
                              rdst[:, ch["xoff"]:ch["xoff"] + 1]).then_inc(s_v, 1)

        @block.tensor
        def _(t):
            t.wait_ge(s_in, 32)
            for si in range(SLOTS):
                t.wait_ge(s_v, slot_end[si])
                if si > 1:
                    t.wait_ge(s_cp, si - 1)
                t.matmul(psum_ts[si % 2][:], sel_t[:], r_ts[si % 2][:],
                         start=True, stop=True).then_inc(s_mm, 1)

        @block.scalar
        def _(sc):
            for si in range(SLOTS):
                sc.wait_ge(s_mm, si + 1)
                if si > 1:
                    sc.wait_ge(s_od, 16 * (si - 1))  # sino buf freed by DMA
                sc.copy(sino_ts[si % 2][:], psum_ts[si % 2][:]).then_inc(s_cp, 1)
                sc.dma_start(out=out_d[si], in_=sino_ts[si % 2][:]
                             ).then_inc(s_od, 16)
            sc.wait_ge(s_od, 16 * SLOTS)

    import concourse.mybir as mybir2
    mybir2.codegen_inst_isa_subclasses(nc)
    _PROG_CACHE["prog"] = nc
    return nc


def kernel(image):
    image = np.asarray(image, np.float32)
    assert image.shape == (BATCH, 1, IMG_SIZE, IMG_SIZE)
    plan = _get_plan()
    nc = _build_program(plan)

    from concourse.bass_utils import run_bass_kernel_spmd

    in_maps = []
    for ci, (var, angles) in enumerate(CORE_SPECS):
        in_maps.append({
            "slab": _build_slab(image, var),
            "idx": plan["core_idx"][ci],
            "w": plan["core_w"][ci],
            "sel": plan["sel"],
        })

    trace = bool(os.environ.get("RADON_TRACE"))
    if trace:
        _install_profhook()
    res = run_bass_kernel_spmd(nc, in_maps, list(range(8)), trace=trace)
    if trace:
        kernel.last_exec_time_ns = res.exec_time_ns

    sino = np.zeros((BATCH, 1, S, N_ANGLES), np.float32)
    for ci in range(8):
        o = res.results[ci]["out"]  # [SLOTS, 2, SXPAD]; slot 0 is dummy
        for si, k in enumerate(plan["core_order"][ci]):
            sino[:, 0, :, k] = o[si + 1, :, :S]
    return sino


def _install_profhook():
    import types
    if "antenv.axon_hooks" in sys.modules:
        return
    try:
        from trn_agent_boot.trn_boot import _ntff_profile_via_ctypes
        hook = _ntff_profile_via_ctypes("/opt/axon/libaxon_pjrt.so")
    except Exception:
        hook = None
    mod = types.ModuleType("antenv.axon_hooks")
    mod._hook = hook
    mod.set_axon_ntff_profile_hook = lambda h: setattr(mod, "_hook", h)
    mod.get_axon_ntff_profile_hook = lambda: mod._hook
    sys.modules["antenv.axon_hooks"] = mod
    import antenv
    antenv.axon_hooks = mod


if __name__ == "__main__":
    img = np.load("/tmp/ref_image.npy")
    out = kernel(image=img)
    exp = np.load("/tmp/ref_expected.npy")
    err = np.linalg.norm(out - exp) / np.linalg.norm(exp)
    print("kernel rel err:", err)
